# revision 67
# baseline (speedup 1.0000x reference)
"""Trainium2 Bass kernel for nn_Net_39230231281866 (dense_cnn).

Network: conv3x3(1->6) -> Taylor-sigmoid -> conv3x3(6->7) -> flatten
         -> fc(4032->128) -> sigmoid -> fc(128->10) -> log_softmax,
batch 8192, data-parallel over 8 NeuronCores (1024 samples/core).

Mapping (v3, fp8 DoubleRow + dense 33-tile conv1 packing):
  * conv2+fc1 folded on the host into one dense GEMM W_comb [128, 4056].
  * conv1 = banded-weight matmul over 33 irregular output tiles (M = 126/
    120/60 rows each, vs 36 rectangular tiles before): the DVE Taylor pass
    is the kernel's pacer and its cost is n_tiles * batch columns, so
    denser M-packing converts directly to wall-clock.
    Tiles: 13 two-row bands x 2 main tiles (21 positions, window 4x13,
    K=52) + 7 leftover tiles (bands' last 5 columns, merged in pairs).
  * Both conv1 and the W_comb GEMM run in fp8e4m3 with
    MatmulPerfMode.DoubleRow (K split into 2 interleaved k-tiles packed
    along the free dim). PSUM accumulation stays fp32. End-to-end
    quantization error ~4e-4 rel (gate is 2e-2).
  * conv1 weights scaled by -8 (= 16 * -1/2); the Taylor denominator
    custom DVE op folds the 1/16 back in: with u = (psum - 8*b1)/16,
    den16(u) = (u(u+1)+1)^2 + u + 2 = u^4+2u^3+3u^2+3u+3  (8/8 v3 stages).
    All tiles start channel-aligned (M % 6 == 0) so one bias vector
    b1[m % 6] serves every tile.
  * All reciprocal ops run on the ScalarE (reciprocal_and_small table),
    writing s as fp8e4 directly; W_comb is scaled by 96 = 64*1.5 (64 keeps
    fp8 weights out of denormals, 1.5 is the den16 normalization), undone
    by the scale of the tail's Exp.
  * Tail sigmoid = 1/(1+exp(-z)) via ACT Exp + DVE reciprocal_approx_fast,
    so the whole tail only needs the natural_log_exp_and_others table:
    exactly 2 ACT table loads per kernel.
  * Input windows are host-packed into per-block (4 tiles at partition
    bases 0/32/64/96) dense [128, 1024] fp8 images; DMA sizes graduate
    (small first blocks) so compute starts ~11us in while the rest
    streams. Start-critical DMAs issue from three different engine queues.
"""

import os
import numpy as np
import ml_dtypes

_B = 8192
_NCORES = 8
_PC = _B // _NCORES
_SLICE = 512
_NSL = _PC // _SLICE

LAST_RESULTS = None

# xwin DMA granularity in blocks (consumption order): small first blocks
# so compute starts early, large later blocks to bound instruction count
_XSIZES = [1, 1, 1, 1, 2, 2, 2, 4, 4]


def _tiles():
    """33 conv1 output tiles, each a channel-aligned run of output
    positions with a compact input window (K <= 64 so half-K <= 32 and
    4 tiles pack into 128 partitions at bases 0/32/64/96)."""
    def mk(pos):
        oys = [p[0] for p in pos]; oxs = [p[1] for p in pos]
        ry0, rx0 = min(oys), min(oxs)
        wr, wc = max(oys) - ry0 + 3, max(oxs) - rx0 + 3
        K = wr * wc
        K2 = (K + 1) // 2
        assert K2 <= 32, (wr, wc)
        return dict(pos=pos, ry0=ry0, rx0=rx0, wr=wr, wc=wc,
                    K=K, K2=K2, M=6 * len(pos))

    mains, leftovers = [], []
    for b in range(13):
        r0 = 2 * b
        pos = [(r0 + rr, c) for c in range(26) for rr in range(2)]
        mains.append(mk(pos[0:21]))
        mains.append(mk(pos[21:42]))
        leftovers.append(pos[42:52])
    ts = list(mains)
    for i in range(6):
        ts.append(mk(leftovers[2 * i] + leftovers[2 * i + 1]))
    ts.append(mk(leftovers[12]))
    assert len(ts) == 33 and sum(t["M"] for t in ts) == 4056
    return ts


# per-slice partition blocks of tiles (4 per block at bases 0/32/64/96)
def _blocks(tiles):
    out, i = [], 0
    while i < len(tiles):
        out.append(list(range(i, min(i + 4, len(tiles)))))
        i += 4
    return out


_GRP = 3  # tiles per psum / den16 / recip / s group


def _z_units(n_tiles):
    """z-GEMM units: ("dr", first_tile, wcpack_col) DoubleRow pairs when
    both tiles share one s group, else ("plain", tile, col) chunks."""
    units, col, t = [], 0, 0
    while t < n_tiles:
        if t + 1 < n_tiles and (t // _GRP) == ((t + 1) // _GRP):
            units.append(("dr", t, col)); col += 256; t += 2
        else:
            units.append(("plain", t, col)); col += 128; t += 1
    return units


def _q8(a):
    return np.asarray(a, np.float32).astype(ml_dtypes.float8_e4m3fn)


def _host_prep(x, w1, b1, w2, b2, fw1, fb1, fw2, fb2):
    x = np.asarray(x, np.float32)
    w1 = np.asarray(w1, np.float32); b1 = np.asarray(b1, np.float32)
    w2 = np.asarray(w2, np.float32); b2 = np.asarray(b2, np.float32)
    fw1 = np.asarray(fw1, np.float32); fb1 = np.asarray(fb1, np.float32)
    fw2 = np.asarray(fw2, np.float32); fb2 = np.asarray(fb2, np.float32)

    tiles = _tiles()
    blocks = _blocks(tiles)
    f8 = ml_dtypes.float8_e4m3fn

    # banded conv1 weights scaled by -8 (psum = -8*conv(x)), DoubleRow
    # interleave, one [K2, 2, 128] stationary per tile at its partition
    # base: w1stat[32g + r, bi*256 + j*128 + m] = wp_t[j*K2 + r, m]
    w1stat = np.zeros((128, 256 * len(blocks)), np.float32)
    for bi, blk in enumerate(blocks):
        for g, t_i in enumerate(blk):
            t = tiles[t_i]
            K2 = t["K2"]
            wp = np.zeros((2 * K2, 128), np.float32)
            for pi_, (oy, ox) in enumerate(t["pos"]):
                for ch in range(6):
                    m = pi_ * 6 + ch
                    for k in range(t["K"]):
                        iy = t["ry0"] + k // t["wc"]
                        ix = t["rx0"] + k % t["wc"]
                        dy, dx = iy - oy, ix - ox
                        if 0 <= dy < 3 and 0 <= dx < 3:
                            wp[k, m] = -8.0 * w1[ch, 0, dy, dx]
            for j in range(2):
                w1stat[32 * g:32 * g + K2,
                       bi * 256 + j * 128:bi * 256 + j * 128 + 128] = \
                    wp[j * K2:(j + 1) * K2, :]

    # fold conv2 + fc1 -> W_comb [128, 6*26*26] (x96 = 64*1.5), b_comb
    fw1r = fw1.reshape(128, 7, 24, 24)
    Wc = np.zeros((128, 6, 26, 26), np.float32)
    for dy in range(3):
        for dx in range(3):
            Wc[:, :, dy:dy + 24, dx:dx + 24] += np.einsum(
                "joyx,oi->jiyx", fw1r, w2[:, :, dy, dx], optimize=True)
    b_comb = fb1 + np.einsum("joyx,o->j", fw1r, b2)
    Wc_flat = (96.0 * Wc.reshape(128, 6 * 26 * 26)).astype(np.float32)

    # W_comb columns. z-units: DoubleRow for tile pairs that live in one
    # 3-tile psum/s group, plain chunks otherwise (see _z_units).
    units = _z_units(len(tiles))
    wcpack = np.zeros((128, 128 * len(tiles)), np.float32)
    for (kind, t0, col0) in units:
        for j in range(2 if kind == "dr" else 1):
            t = tiles[t0 + j]
            # m ordering within tile: pos-major, channel fastest
            rows = [(ch * 26 + oy) * 26 + ox
                    for (oy, ox) in t["pos"] for ch in range(6)]
            wcpack[:t["M"], col0 + j * 128:col0 + (j + 1) * 128] = \
                Wc_flat[:, rows].T

    # f32 const blob: [bias1 | bcombN | fb2r] ([128, 42])
    bias1 = np.array([-8.0 * b1[m % 6] for m in range(128)],
                     np.float32).reshape(128, 1)
    cb32 = np.concatenate(
        [bias1, (-b_comb).reshape(128, 1).astype(np.float32),
         np.tile(fb2.reshape(1, 10), (128, 4)).astype(np.float32)], axis=1)

    consts = dict(
        w1stat=_q8(w1stat), wcpack=_q8(wcpack),
        cb32=np.ascontiguousarray(cb32),
        fw2t=np.ascontiguousarray(fw2.T).astype(np.float16),   # [128, 10]
    )

    # per-tile window data [2, K2, B] fp8 (k-tile-interleaved, zero-padded)
    x_pm = _q8(x.reshape(_B, 784).T)                           # [784, B]
    tile_wins = []
    for t in tiles:
        K2 = t["K2"]
        rows = ((np.arange(t["wr"])[:, None] + t["ry0"]) * 28 +
                (np.arange(t["wc"])[None, :] + t["rx0"])).reshape(-1)
        w = np.zeros((2 * K2, _B), f8)
        w[:t["K"], :] = x_pm[rows, :]
        tile_wins.append(w.reshape(2, K2, _B))
    return tile_wins, consts, tiles


def _register_taylor_den16s():
    import concourse.dve_ops as dve_ops
    name = "TAYLOR_DEN16S_ANT"
    if name in dve_ops._SUB_OPCODE_FOR_NAME:
        return next(o for o in dve_ops.OPS if o.name == name)
    from concourse.dve_spec import Spec, Src0, C0, C1, C2, One, lower, sq
    from concourse.dve_uop import DveOpSpec

    # u = (in0 + s0) * s1;  out = (u*(u+1)+1)^2 + u + imm2
    #   == u^4 + 2u^3 + 3u^2 + 3u + (1 + imm2)   (imm2 = 2 -> den16)
    u = (Src0 + C0) * C1
    body = sq(u * (u + One) + One) + u + C2

    def _ref(in0, in1, s0, s1, imm2):
        uu = (in0.astype(np.float32) + s0) * s1
        return (uu * (uu + 1.0) + 1.0) ** 2 + uu + imm2

    spec = Spec(body=body, reference=_ref)
    row = max(dve_ops._SUB_OPCODE_FOR_NAME.values()) + 1
    assert row < 0x20
    shas = {ver: DveOpSpec(name=name, opcode=row, uops=lower(spec, ver=ver),
                           rd1_en=False).sha(ver)
            for ver in ("v3", "v4")}
    op = dve_ops.DveOp(name, spec, subdim=False, uops_sha=shas)
    dve_ops.OPS.append(op)
    dve_ops.CUSTOM_DVE_SPECS[op.name] = op.spec
    dve_ops._SUB_OPCODE_FOR_NAME[op.name] = row
    return op


def _pin_act_tables():
    """Pin Copy -> reciprocal_and_small and Exp/Ln ->
    natural_log_exp_and_others so the kernel loads exactly 2 ACT tables."""
    import concourse.bacc as bacc
    import concourse.mybir as mybir
    if getattr(bacc, "_ant_tables_pinned", False):
        return
    orig = bacc.get_activation_tables
    AF = mybir.ActivationFunctionType

    def patched(arch):
        tabs = {k: set(v) for k, v in orig(arch).items()}
        for name, fns in tabs.items():
            if name != "natural_log_exp_and_others":
                fns.discard(AF.Exp)
                fns.discard(AF.Ln)
            if name != "reciprocal_and_small":
                fns.discard(AF.Copy)
        return tabs

    bacc.get_activation_tables = patched
    bacc._ant_tables_pinned = True


def _act_raw(nc, out, in_, func, bias=0.0, scale=1.0):
    """Emit InstActivation directly (used for Reciprocal, which the
    nc.scalar.activation wrapper refuses; measured ~1.2e-5 rel err)."""
    import concourse.mybir as mybir
    eng = nc.scalar
    inputs = [eng.lower_ap(in_)]
    for arg in (bias, scale, 0.0):
        inputs.append(mybir.ImmediateValue(dtype=mybir.dt.float32,
                                           value=float(arg)))
    return eng.add_instruction(mybir.InstActivation(
        name=nc.get_next_instruction_name(), func=func, ins=inputs,
        outs=[eng.lower_ap(out)]))


def _build_program(tiles):
    import concourse.bacc as bacc
    import concourse.mybir as mybir
    from concourse.tile import TileContext
    from concourse.tile_rust import add_dep_helper
    from concourse.alu_op_type import AluOpType
    from concourse.dve_ops import RECIP_APPROX_FAST_CONSTS as RC
    from concourse.dve_ops import RECIPROCAL_APPROX_FAST

    f32 = mybir.dt.float32
    f16 = mybir.dt.float16
    f8 = mybir.dt.float8e4
    AF = mybir.ActivationFunctionType
    DR = mybir.MatmulPerfMode.DoubleRow
    den_op = _register_taylor_den16s()
    _pin_act_tables()

    nc = bacc.Bacc()
    blocks = _blocks(tiles)
    n_bps = len(blocks)                      # blocks per slice (9)
    n_tiles = len(tiles)
    zunits = _z_units(n_tiles)
    n_grp = (n_tiles + _GRP - 1) // _GRP     # psum/s groups per slice (11)
    xwin = nc.declare_dram_parameter("xwin", [128, _NSL * n_bps * 1024], f8,
                                     isOutput=False)
    w1stat_d = nc.declare_dram_parameter("w1stat", [128, 256 * n_bps], f8,
                                         isOutput=False)
    wcpack_d = nc.declare_dram_parameter("wcpack", [128, 128 * n_tiles], f8,
                                         isOutput=False)
    cb32_d = nc.declare_dram_parameter("cb32", [128, 42], f32, isOutput=False)
    fw2t_d = nc.declare_dram_parameter("fw2t", [128, 10], f16, isOutput=False)
    out_d = nc.declare_dram_parameter("out", [_PC, 10], f32, isOutput=True)
    n_wc = 128 * n_tiles

    with TileContext(nc) as tc:
        with (
            tc.tile_pool(name="const", bufs=1) as cpool,
            tc.tile_pool(name="xq", bufs=5) as xpool,
            tc.tile_pool(name="q", bufs=3) as qpool,
            tc.tile_pool(name="s", bufs=3) as spool,
            tc.tile_pool(name="work", bufs=3) as wpool,
            tc.tile_pool(name="cps", bufs=2, space="PSUM") as cps,
            tc.tile_pool(name="zps", bufs=2, space="PSUM") as zps,
        ):
            # DMA order tuned for pipeline start: tiny cb32 (DVE preamble
            # reads it) and w1stat first (on side queues), first input
            # blocks, then wcpack split in two for parallel transfer (first
            # z matmul needs it ~15us in), then the rest of the inputs.
            POOLE = mybir.EngineType.Pool
            ACTE = mybir.EngineType.Activation
            cb32_sb = cpool.tile_from(cb32_d[:], name="cb32_sb",
                                      forced_dma_engine=POOLE)
            biasp_sb = cb32_sb[:, 0:1]
            bcombN_sb = cb32_sb[:, 1:2]
            fb2r_sb = cb32_sb[:, 2:42]
            # w1stat split across two queues (295KB on one queue would gate
            # the first conv1 by ~6us); block 0's columns go first so the
            # first conv1 is gated only by the first input half-DMA
            w1stat_sb = cpool.tile([128, 256 * n_bps], f8, tag="w1s",
                                   name="w1stat_sb")
            h1 = 256 * (n_bps // 2)
            nc.scalar.dma_start(out=w1stat_sb[:, 0:256],
                                in_=w1stat_d[:, 0:256])
            nc.scalar.dma_start(out=w1stat_sb[:, 256:h1],
                                in_=w1stat_d[:, 256:h1])
            nc.gpsimd.dma_start(out=w1stat_sb[:, h1:256 * n_bps],
                                in_=w1stat_d[:, h1:256 * n_bps])

            n_blk = _NSL * n_bps
            sizes = []
            while sum(sizes) < n_blk:
                nb = _XSIZES[len(sizes)] if len(sizes) < len(_XSIZES) else 4
                sizes.append(min(nb, n_blk - sum(sizes)))
            xdma = []
            b0 = 0
            for di, nb in enumerate(sizes):
                t = xpool.tile([128, 1024 * nb], f8, tag=f"xq_{di}",
                               name=f"xq{b0}", bufs=1)
                xdma.append((b0, nb, t))
                b0 += nb

            def xissue(di):
                bb, nb, t = xdma[di]
                nc.sync.dma_start(out=t,
                                  in_=xwin[:, bb * 1024:(bb + nb) * 1024])

            # first block split by partition halves: the first conv1 pair
            # only reads partitions 0-63, so it can start on the half-DMA
            # group 0 (tiles 0-2 at bases 0/32/64, K2<=26) is staggered
            # per-tile: three 32-row DMAs so tile 0's conv1/den16 start on
            # a 32KB transfer instead of the full block
            t0_ = xdma[0][2]
            nc.sync.dma_start(out=t0_[0:32, :], in_=xwin[0:32, 0:1024])
            nc.sync.dma_start(out=t0_[32:64, :], in_=xwin[32:64, 0:1024])
            nc.sync.dma_start(out=t0_[64:96, :], in_=xwin[64:96, 0:1024])
            nc.sync.dma_start(out=t0_[96:128, :], in_=xwin[96:128, 0:1024])
            xissue(1)
            wcpack_sb = cpool.tile([128, n_wc], f8, tag="wcp",
                                   name="wcpack_sb")
            nc.sync.dma_start(out=wcpack_sb[:, 0:n_wc // 2],
                              in_=wcpack_d[:, 0:n_wc // 2])
            nc.sync.dma_start(out=wcpack_sb[:, n_wc // 2:n_wc],
                              in_=wcpack_d[:, n_wc // 2:n_wc])
            fw2t_sb = cpool.tile_from(fw2t_d[:], name="fw2t_sb")
            for di in range(2, len(sizes)):
                xissue(di)

            def quad_ap(sl, bi):
                blk = sl * n_bps + bi
                for (bb, nb, t) in xdma:
                    if bb <= blk < bb + nb:
                        return t[:, (blk - bb) * 1024:(blk - bb) * 1024 + 1024]
                raise AssertionError(blk)

            # z psum tiles (also the dummy-matmul target for the
            # single-sync-wait preamble)
            zs = [zps.tile([128, _SLICE], f32, tag="z", name=f"z{sl}")
                  for sl in range(_NSL)]

            # single-sync-wait rule: pre-observe PE-read const queues with
            # dummy 1-col matmuls; DVE/ACT-read consts with dummy touches.
            nc.tensor.matmul(zs[0][0:128, 0:1], w1stat_sb[0:26, 0:128],
                             w1stat_sb[0:26, 0:1], start=True, stop=True)
            dvescr = wpool.tile([128, 44], f32, tag="dvescr", name="dvescr",
                                bufs=1)
            nc.vector.tensor_copy(out=dvescr[:, 0:1], in_=biasp_sb[:])
            nc.vector.tensor_copy(out=dvescr[:, 4:44], in_=fb2r_sb[:])
            actscr = wpool.tile([128, 1], f32, tag="actscr", name="actscr",
                                bufs=1)
            nc.scalar.copy(out=actscr[:], in_=bcombN_sb[:])

            recip_insts = []
            for sl in range(_NSL):
                stiles = {}      # group -> s tile
                zu_next = 0
                for gi in range(n_grp):
                    gtiles = list(range(gi * _GRP,
                                        min((gi + 1) * _GRP, n_tiles)))
                    ng_t = len(gtiles)
                    cp = cps.tile([128, _GRP * _SLICE], f32, tag="cp",
                                  name=f"cp{sl}_{gi}")
                    for j, ti in enumerate(gtiles):
                        t = tiles[ti]
                        K2 = t["K2"]
                        bi, g = divmod(ti, 4)
                        quad = quad_ap(sl, bi)
                        rhs = quad[32 * g:32 * g + K2, :].rearrange(
                            "p (two n) -> p two n", two=2)
                        lhsT = w1stat_sb[
                            32 * g:32 * g + K2,
                            bi * 256:bi * 256 + 256].rearrange(
                            "p (two m) -> p two m", two=2)
                        nc.tensor.matmul(
                            cp[:, j * _SLICE:(j + 1) * _SLICE], lhsT,
                            rhs, start=True, stop=True, perf_mode=DR,
                            tile_position=(32 * g, 0))
                    q = qpool.tile([128, _GRP * _SLICE], f32, tag="q",
                                   name=f"q{sl}_{gi}")
                    s = spool.tile([128, _GRP * _SLICE], f8, tag="s",
                                   name=f"s{sl}_{gi}")
                    stiles[gi] = s
                    if sl == 0 and gi == 0:
                        # per-tile ops so den16 starts on the first 32-row
                        # input DMA instead of the whole first block
                        for j in range(ng_t):
                            nc.vector._custom_dve(
                                den_op,
                                out=q[:, j * _SLICE:(j + 1) * _SLICE],
                                in0=cp[:, j * _SLICE:(j + 1) * _SLICE],
                                s0=biasp_sb[0:128, 0:1],
                                s1=1.0 / 16.0, imm2=2.0)
                    else:
                        nc.vector._custom_dve(
                            den_op, out=q[:, 0:ng_t * _SLICE],
                            in0=cp[:, 0:ng_t * _SLICE],
                            s0=biasp_sb[0:128, 0:1], s1=1.0 / 16.0, imm2=2.0)
                    if sl == 0 and gi == 0:
                        # deferred const-queue dummies (single-sync-wait):
                        # emitted after group 0's conv1 so they park in the
                        # PE wait queue while their (later) DMAs land.
                        nc.tensor.matmul(zs[0][0:128, 0:1],
                                         wcpack_sb[0:128, 0:128],
                                         wcpack_sb[0:128, 0:1],
                                         start=True, stop=True)
                        nc.tensor.matmul(zs[0][0:10, 0:1],
                                         fw2t_sb[0:128, 0:10],
                                         fw2t_sb[0:128, 0:1],
                                         start=True, stop=True)
                    if sl == _NSL - 1 and gi == n_grp - 1:
                        # final group's reciprocal on the (otherwise idle)
                        # DVE so the ACT queue can start the table switch
                        # and tail while this group finishes
                        nc.vector._custom_dve(
                            RECIPROCAL_APPROX_FAST,
                            out=s[:, 0:ng_t * _SLICE],
                            in0=q[:, 0:ng_t * _SLICE],
                            s0=RC["s0"], s1=RC["s1"], imm2=RC["imm2"])
                    else:
                        ri = _act_raw(nc, s[:, 0:ng_t * _SLICE],
                                      q[:, 0:ng_t * _SLICE], AF.Reciprocal)
                        recip_insts.append(ri)
                    # z units whose tiles are all covered by groups <= gi
                    while zu_next < len(zunits):
                        kind, t0, col0 = zunits[zu_next]
                        t_last = t0 + (1 if kind == "dr" else 0)
                        if t_last > gtiles[-1]:
                            break
                        sg = stiles[t0 // _GRP]
                        o0 = (t0 % _GRP) * _SLICE
                        if kind == "dr":
                            lhsT = wcpack_sb[
                                :, col0:col0 + 256].rearrange(
                                "p (two m) -> p two m", two=2)
                            rhs = sg[:, o0:o0 + 1024].rearrange(
                                "p (two n) -> p two n", two=2)
                            nc.tensor.matmul(zs[sl], lhsT, rhs,
                                             start=(t0 == 0), stop=False,
                                             perf_mode=DR)
                        else:
                            lhsT = wcpack_sb[:, col0:col0 + 128]
                            rhs = sg[:, o0:o0 + _SLICE]
                            nc.tensor.matmul(
                                zs[sl], lhsT, rhs, start=(t0 == 0),
                                stop=(t_last == n_tiles - 1))
                        zu_next += 1

            # ---- tail: sigmoid via Exp + fast-reciprocal, fc2, log_softmax.
            # (no max-sub: |logits| < 12, exp cannot overflow fp32.)
            # NOTE: must stay after ALL recips - interleaving tail ACT ops
            # with recips thrashes the ACT table sets (measured 8 loads).
            last_recip = recip_insts[-1]
            for sl in range(_NSL):
                e = wpool.tile([128, _SLICE], f32, tag="e", name=f"e{sl}")
                ei = nc.scalar.activation(e, zs[sl], AF.Exp, bias=bcombN_sb[:],
                                          scale=-1.0 / 64.0)
                add_dep_helper(ei.ins, last_recip.ins, sync=False,
                               reason="keep tail ACT after recips (table sets)")
                t1 = wpool.tile([128, _SLICE], f32, tag="t1", name=f"t1{sl}")
                nc.vector.tensor_scalar_add(t1, e, 1.0)
                h = wpool.tile([128, _SLICE], f16, tag="h", name=f"h{sl}")
                nc.vector._custom_dve(RECIPROCAL_APPROX_FAST, out=h, in0=t1,
                                      s0=RC["s0"], s1=RC["s1"], imm2=RC["imm2"])
                ng = _SLICE // 128
                # fc2 psum borrows the cps pool (all 8 banks are spoken for;
                # the cp rotation is idle by the time the tail runs)
                fp = cps.tile([128, _GRP * _SLICE], f32, tag="cp",
                              name=f"fp{sl}")
                for g in range(ng):
                    nc.tensor.matmul(fp[:, g * 10:(g + 1) * 10],
                                     h[:, g * 128:(g + 1) * 128], fw2t_sb[:],
                                     start=True, stop=True)
                lg = wpool.tile([128, 10 * ng], f32, tag="lg", name=f"lg{sl}")
                nc.vector.tensor_tensor(out=lg, in0=fp[:, 0:10 * ng],
                                        in1=fb2r_sb[:, 0:10 * ng],
                                        op=AluOpType.add)
                e2 = wpool.tile([128, 10 * ng], f32, tag="e2", name=f"e2{sl}")
                e2i = nc.scalar.activation(e2, lg, AF.Exp)
                add_dep_helper(e2i.ins, last_recip.ins, sync=False,
                               reason="keep tail ACT after recips (table sets)")
                ssum = wpool.tile([128, ng], f32, tag="ss", name=f"ss{sl}")
                nc.vector.tensor_reduce(
                    ssum, e2.rearrange("p (g k) -> p g k", k=10),
                    axis=mybir.AxisListType.X, op=AluOpType.add)
                lns = wpool.tile([128, ng], f32, tag="ls", name=f"ls{sl}")
                li = nc.scalar.activation(lns, ssum, AF.Ln)
                add_dep_helper(li.ins, last_recip.ins, sync=False,
                               reason="keep tail ACT after recips (table sets)")
                ot = wpool.tile([128, 10 * ng], f32, tag="ot", name=f"ot{sl}")
                nc.vector.tensor_tensor(
                    out=ot.rearrange("p (g k) -> p g k", k=10),
                    in0=lg.rearrange("p (g k) -> p g k", k=10),
                    in1=lns.rearrange("p (g o) -> p g o", o=1).to_broadcast(
                        [128, ng, 10]),
                    op=AluOpType.subtract)
                orow = sl * _SLICE
                oeng = nc.sync if sl == 0 else nc.scalar
                oeng.dma_start(
                    out=out_d[orow:orow + _SLICE, :].rearrange(
                        "(g p) k -> p g k", p=128),
                    in_=ot.rearrange("p (g k) -> p g k", k=10))
    nc.compile()
    return nc


_PROGRAM_CACHE = {}


def kernel(x, w1, b1, w2, b2, fw1, fb1, fw2, fb2):
    global LAST_RESULTS
    tile_wins, consts, tiles = _host_prep(x, w1, b1, w2, b2, fw1, fb1, fw2, fb2)

    if "nc" not in _PROGRAM_CACHE:
        _PROGRAM_CACHE["nc"] = _build_program(tiles)
    nc = _PROGRAM_CACHE["nc"]

    f8 = ml_dtypes.float8_e4m3fn
    blocks = _blocks(tiles)
    n_bps = len(blocks)
    shared = {k: consts[k] for k in ("w1stat", "wcpack", "cb32", "fw2t")}
    in_maps = []
    for c in range(_NCORES):
        m = dict(shared)
        blob = np.zeros((128, _NSL * n_bps * 1024), f8)
        for sl in range(_NSL):
            for bi, blk in enumerate(blocks):
                col0 = (sl * n_bps + bi) * 1024
                for g, t_i in enumerate(blk):
                    t = tiles[t_i]
                    K2 = t["K2"]
                    w = tile_wins[t_i]            # [2, K2, B]
                    c0 = c * _PC + sl * _SLICE
                    for j in range(2):
                        blob[32 * g:32 * g + K2,
                             col0 + j * _SLICE:col0 + (j + 1) * _SLICE] = \
                            w[j, :, c0:c0 + _SLICE]
        m["xwin"] = blob
        in_maps.append(m)

    from concourse.bass_utils import run_bass_kernel_spmd
    trace = bool(int(os.environ.get("BASS_KERNEL_TRACE", "0")))
    res = run_bass_kernel_spmd(nc, in_maps, core_ids=list(range(_NCORES)),
                               trace=trace)
    LAST_RESULTS = res
    return np.concatenate([r["out"] for r in res.results], axis=0)


# revision 69
# speedup vs baseline: 1.0275x; 1.0275x over previous
"""Trainium2 Bass kernel for nn_Net_39230231281866 (dense_cnn).

Network: conv3x3(1->6) -> Taylor-sigmoid -> conv3x3(6->7) -> flatten
         -> fc(4032->128) -> sigmoid -> fc(128->10) -> log_softmax,
batch 8192, data-parallel over 8 NeuronCores (1024 samples/core).

Mapping (v3, fp8 DoubleRow + dense 33-tile conv1 packing):
  * conv2+fc1 folded on the host into one dense GEMM W_comb [128, 4056].
  * conv1 = banded-weight matmul over 33 irregular output tiles (M = 126/
    120/60 rows each, vs 36 rectangular tiles before): the DVE Taylor pass
    is the kernel's pacer and its cost is n_tiles * batch columns, so
    denser M-packing converts directly to wall-clock.
    Tiles: 13 two-row bands x 2 main tiles (21 positions, window 4x13,
    K=52) + 7 leftover tiles (bands' last 5 columns, merged in pairs).
  * Both conv1 and the W_comb GEMM run in fp8e4m3 with
    MatmulPerfMode.DoubleRow (K split into 2 interleaved k-tiles packed
    along the free dim). PSUM accumulation stays fp32. End-to-end
    quantization error ~4e-4 rel (gate is 2e-2).
  * conv1 weights scaled by -8 (= 16 * -1/2); the Taylor denominator
    custom DVE op folds the 1/16 back in: with u = (psum - 8*b1)/16,
    den16(u) = (u(u+1)+1)^2 + u + 2 = u^4+2u^3+3u^2+3u+3  (8/8 v3 stages).
    All tiles start channel-aligned (M % 6 == 0) so one bias vector
    b1[m % 6] serves every tile.
  * All reciprocal ops run on the ScalarE (reciprocal_and_small table),
    writing s as fp8e4 directly; W_comb is scaled by 96 = 64*1.5 (64 keeps
    fp8 weights out of denormals, 1.5 is the den16 normalization), undone
    by the scale of the tail's Exp.
  * Tail sigmoid = 1/(1+exp(-z)) via ACT Exp + DVE reciprocal_approx_fast,
    so the whole tail only needs the natural_log_exp_and_others table:
    exactly 2 ACT table loads per kernel.
  * Input windows are host-packed into per-block (4 tiles at partition
    bases 0/32/64/96) dense [128, 1024] fp8 images; DMA sizes graduate
    (small first blocks) so compute starts ~11us in while the rest
    streams. Start-critical DMAs issue from three different engine queues.
"""

import os
import numpy as np
import ml_dtypes

_B = 8192
_NCORES = 8
_PC = _B // _NCORES
_SLICE = 512
_NSL = _PC // _SLICE

LAST_RESULTS = None

# xwin DMA granularity in blocks (consumption order): small first blocks
# so compute starts early, large later blocks to bound instruction count
_XSIZES = [1, 1, 1, 1, 2, 2, 2, 4, 4]


def _tiles():
    """33 conv1 output tiles, each a channel-aligned run of output
    positions with a compact input window (K <= 64 so half-K <= 32 and
    4 tiles pack into 128 partitions at bases 0/32/64/96)."""
    def mk(pos):
        oys = [p[0] for p in pos]; oxs = [p[1] for p in pos]
        ry0, rx0 = min(oys), min(oxs)
        wr, wc = max(oys) - ry0 + 3, max(oxs) - rx0 + 3
        K = wr * wc
        K2 = (K + 1) // 2
        assert K2 <= 32, (wr, wc)
        return dict(pos=pos, ry0=ry0, rx0=rx0, wr=wr, wc=wc,
                    K=K, K2=K2, M=6 * len(pos))

    mains, leftovers = [], []
    for b in range(13):
        r0 = 2 * b
        pos = [(r0 + rr, c) for c in range(26) for rr in range(2)]
        mains.append(mk(pos[0:21]))
        mains.append(mk(pos[21:42]))
        leftovers.append(pos[42:52])
    ts = list(mains)
    for i in range(6):
        ts.append(mk(leftovers[2 * i] + leftovers[2 * i + 1]))
    ts.append(mk(leftovers[12]))
    assert len(ts) == 33 and sum(t["M"] for t in ts) == 4056
    return ts


# per-slice partition blocks of tiles (4 per block at bases 0/32/64/96)
def _blocks(tiles):
    out, i = [], 0
    while i < len(tiles):
        out.append(list(range(i, min(i + 4, len(tiles)))))
        i += 4
    return out


_GRP = 3  # tiles per psum / den16 / recip / s group


def _z_units(n_tiles):
    """z-GEMM units: ("dr", first_tile, wcpack_col) DoubleRow pairs when
    both tiles share one s group, else ("plain", tile, col) chunks."""
    units, col, t = [], 0, 0
    while t < n_tiles:
        if t + 1 < n_tiles and (t // _GRP) == ((t + 1) // _GRP):
            units.append(("dr", t, col)); col += 256; t += 2
        else:
            units.append(("plain", t, col)); col += 128; t += 1
    return units


def _q8(a):
    return np.asarray(a, np.float32).astype(ml_dtypes.float8_e4m3fn)


def _host_prep(x, w1, b1, w2, b2, fw1, fb1, fw2, fb2):
    x = np.asarray(x, np.float32)
    w1 = np.asarray(w1, np.float32); b1 = np.asarray(b1, np.float32)
    w2 = np.asarray(w2, np.float32); b2 = np.asarray(b2, np.float32)
    fw1 = np.asarray(fw1, np.float32); fb1 = np.asarray(fb1, np.float32)
    fw2 = np.asarray(fw2, np.float32); fb2 = np.asarray(fb2, np.float32)

    tiles = _tiles()
    blocks = _blocks(tiles)
    f8 = ml_dtypes.float8_e4m3fn

    # banded conv1 weights scaled by -8 (psum = -8*conv(x)), DoubleRow
    # interleave, one [K2, 2, 128] stationary per tile at its partition
    # base: w1stat[32g + r, bi*256 + j*128 + m] = wp_t[j*K2 + r, m]
    w1stat = np.zeros((128, 256 * len(blocks)), np.float32)
    for bi, blk in enumerate(blocks):
        for g, t_i in enumerate(blk):
            t = tiles[t_i]
            K2 = t["K2"]
            wp = np.zeros((2 * K2, 128), np.float32)
            for pi_, (oy, ox) in enumerate(t["pos"]):
                for ch in range(6):
                    m = pi_ * 6 + ch
                    for k in range(t["K"]):
                        iy = t["ry0"] + k // t["wc"]
                        ix = t["rx0"] + k % t["wc"]
                        dy, dx = iy - oy, ix - ox
                        if 0 <= dy < 3 and 0 <= dx < 3:
                            wp[k, m] = -8.0 * w1[ch, 0, dy, dx]
            for j in range(2):
                w1stat[32 * g:32 * g + K2,
                       bi * 256 + j * 128:bi * 256 + j * 128 + 128] = \
                    wp[j * K2:(j + 1) * K2, :]

    # fold conv2 + fc1 -> W_comb [128, 6*26*26] (x96 = 64*1.5), b_comb
    fw1r = fw1.reshape(128, 7, 24, 24)
    Wc = np.zeros((128, 6, 26, 26), np.float32)
    for dy in range(3):
        for dx in range(3):
            Wc[:, :, dy:dy + 24, dx:dx + 24] += np.einsum(
                "joyx,oi->jiyx", fw1r, w2[:, :, dy, dx], optimize=True)
    b_comb = fb1 + np.einsum("joyx,o->j", fw1r, b2)
    Wc_flat = (96.0 * Wc.reshape(128, 6 * 26 * 26)).astype(np.float32)

    # W_comb columns. z-units: DoubleRow for tile pairs that live in one
    # 3-tile psum/s group, plain chunks otherwise (see _z_units).
    units = _z_units(len(tiles))
    wcpack = np.zeros((128, 128 * len(tiles)), np.float32)
    for (kind, t0, col0) in units:
        for j in range(2 if kind == "dr" else 1):
            t = tiles[t0 + j]
            # m ordering within tile: pos-major, channel fastest
            rows = [(ch * 26 + oy) * 26 + ox
                    for (oy, ox) in t["pos"] for ch in range(6)]
            wcpack[:t["M"], col0 + j * 128:col0 + (j + 1) * 128] = \
                Wc_flat[:, rows].T

    # f32 const blob: [bias1 | bcombN | fb2r] ([128, 42])
    bias1 = np.array([-8.0 * b1[m % 6] for m in range(128)],
                     np.float32).reshape(128, 1)
    cb32 = np.concatenate(
        [bias1, (-b_comb).reshape(128, 1).astype(np.float32),
         np.tile(fb2.reshape(1, 10), (128, 4)).astype(np.float32)], axis=1)

    consts = dict(
        w1stat=_q8(w1stat), wcpack=_q8(wcpack),
        cb32=np.ascontiguousarray(cb32),
        fw2t=np.ascontiguousarray(fw2.T).astype(np.float16),   # [128, 10]
    )

    # per-tile window data [2, K2, B] fp8 (k-tile-interleaved, zero-padded)
    x_pm = _q8(x.reshape(_B, 784).T)                           # [784, B]
    tile_wins = []
    for t in tiles:
        K2 = t["K2"]
        rows = ((np.arange(t["wr"])[:, None] + t["ry0"]) * 28 +
                (np.arange(t["wc"])[None, :] + t["rx0"])).reshape(-1)
        w = np.zeros((2 * K2, _B), f8)
        w[:t["K"], :] = x_pm[rows, :]
        tile_wins.append(w.reshape(2, K2, _B))
    return tile_wins, consts, tiles


def _register_taylor_den16s():
    import concourse.dve_ops as dve_ops
    name = "TAYLOR_DEN16S_ANT"
    if name in dve_ops._SUB_OPCODE_FOR_NAME:
        return next(o for o in dve_ops.OPS if o.name == name)
    from concourse.dve_spec import Spec, Src0, C0, C1, C2, One, lower, sq
    from concourse.dve_uop import DveOpSpec

    # u = (in0 + s0) * s1;  out = (u*(u+1)+1)^2 + u + imm2
    #   == u^4 + 2u^3 + 3u^2 + 3u + (1 + imm2)   (imm2 = 2 -> den16)
    u = (Src0 + C0) * C1
    body = sq(u * (u + One) + One) + u + C2

    def _ref(in0, in1, s0, s1, imm2):
        uu = (in0.astype(np.float32) + s0) * s1
        return (uu * (uu + 1.0) + 1.0) ** 2 + uu + imm2

    spec = Spec(body=body, reference=_ref)
    row = max(dve_ops._SUB_OPCODE_FOR_NAME.values()) + 1
    assert row < 0x20
    shas = {ver: DveOpSpec(name=name, opcode=row, uops=lower(spec, ver=ver),
                           rd1_en=False).sha(ver)
            for ver in ("v3", "v4")}
    op = dve_ops.DveOp(name, spec, subdim=False, uops_sha=shas)
    dve_ops.OPS.append(op)
    dve_ops.CUSTOM_DVE_SPECS[op.name] = op.spec
    dve_ops._SUB_OPCODE_FOR_NAME[op.name] = row
    return op


def _pin_act_tables():
    """Pin Copy -> reciprocal_and_small and Exp/Ln ->
    natural_log_exp_and_others so the kernel loads exactly 2 ACT tables."""
    import concourse.bacc as bacc
    import concourse.mybir as mybir
    if getattr(bacc, "_ant_tables_pinned", False):
        return
    orig = bacc.get_activation_tables
    AF = mybir.ActivationFunctionType

    def patched(arch):
        tabs = {k: set(v) for k, v in orig(arch).items()}
        for name, fns in tabs.items():
            if name != "natural_log_exp_and_others":
                fns.discard(AF.Exp)
                fns.discard(AF.Ln)
            if name != "reciprocal_and_small":
                fns.discard(AF.Copy)
        return tabs

    bacc.get_activation_tables = patched
    bacc._ant_tables_pinned = True


def _act_raw(nc, out, in_, func, bias=0.0, scale=1.0):
    """Emit InstActivation directly (used for Reciprocal, which the
    nc.scalar.activation wrapper refuses; measured ~1.2e-5 rel err)."""
    import concourse.mybir as mybir
    eng = nc.scalar
    inputs = [eng.lower_ap(in_)]
    for arg in (bias, scale, 0.0):
        inputs.append(mybir.ImmediateValue(dtype=mybir.dt.float32,
                                           value=float(arg)))
    return eng.add_instruction(mybir.InstActivation(
        name=nc.get_next_instruction_name(), func=func, ins=inputs,
        outs=[eng.lower_ap(out)]))


def _build_program(tiles):
    import concourse.bacc as bacc
    import concourse.mybir as mybir
    from concourse.tile import TileContext
    from concourse.tile_rust import add_dep_helper
    from concourse.alu_op_type import AluOpType
    from concourse.dve_ops import RECIP_APPROX_FAST_CONSTS as RC
    from concourse.dve_ops import RECIPROCAL_APPROX_FAST

    f32 = mybir.dt.float32
    f16 = mybir.dt.float16
    f8 = mybir.dt.float8e4
    AF = mybir.ActivationFunctionType
    DR = mybir.MatmulPerfMode.DoubleRow
    den_op = _register_taylor_den16s()
    _pin_act_tables()

    nc = bacc.Bacc()
    blocks = _blocks(tiles)
    n_bps = len(blocks)                      # blocks per slice (9)
    n_tiles = len(tiles)
    zunits = _z_units(n_tiles)
    n_grp = (n_tiles + _GRP - 1) // _GRP     # psum/s groups per slice (11)
    xwin = nc.declare_dram_parameter("xwin", [128, _NSL * n_bps * 1024], f8,
                                     isOutput=False)
    w1stat_d = nc.declare_dram_parameter("w1stat", [128, 256 * n_bps], f8,
                                         isOutput=False)
    wcpack_d = nc.declare_dram_parameter("wcpack", [128, 128 * n_tiles], f8,
                                         isOutput=False)
    cb32_d = nc.declare_dram_parameter("cb32", [128, 42], f32, isOutput=False)
    fw2t_d = nc.declare_dram_parameter("fw2t", [128, 10], f16, isOutput=False)
    out_d = nc.declare_dram_parameter("out", [_PC, 10], f32, isOutput=True)
    n_wc = 128 * n_tiles

    with TileContext(nc) as tc:
        with (
            tc.tile_pool(name="const", bufs=1) as cpool,
            tc.tile_pool(name="xq", bufs=5) as xpool,
            tc.tile_pool(name="q", bufs=3) as qpool,
            tc.tile_pool(name="s", bufs=3) as spool,
            tc.tile_pool(name="work", bufs=3) as wpool,
            tc.tile_pool(name="cps", bufs=2, space="PSUM") as cps,
            tc.tile_pool(name="zps", bufs=2, space="PSUM") as zps,
        ):
            # DMA order tuned for pipeline start: tiny cb32 (DVE preamble
            # reads it) and w1stat first (on side queues), first input
            # blocks, then wcpack split in two for parallel transfer (first
            # z matmul needs it ~15us in), then the rest of the inputs.
            POOLE = mybir.EngineType.Pool
            ACTE = mybir.EngineType.Activation
            cb32_sb = cpool.tile_from(cb32_d[:], name="cb32_sb",
                                      forced_dma_engine=POOLE)
            biasp_sb = cb32_sb[:, 0:1]
            bcombN_sb = cb32_sb[:, 1:2]
            fb2r_sb = cb32_sb[:, 2:42]
            # w1stat split across two queues (295KB on one queue would gate
            # the first conv1 by ~6us); block 0's columns go first so the
            # first conv1 is gated only by the first input half-DMA
            w1stat_sb = cpool.tile([128, 256 * n_bps], f8, tag="w1s",
                                   name="w1stat_sb")
            h1 = 256 * (n_bps // 2)
            nc.scalar.dma_start(out=w1stat_sb[:, 0:256],
                                in_=w1stat_d[:, 0:256])
            nc.scalar.dma_start(out=w1stat_sb[:, 256:h1],
                                in_=w1stat_d[:, 256:h1])
            nc.gpsimd.dma_start(out=w1stat_sb[:, h1:256 * n_bps],
                                in_=w1stat_d[:, h1:256 * n_bps])

            n_blk = _NSL * n_bps
            sizes = []
            while sum(sizes) < n_blk:
                nb = _XSIZES[len(sizes)] if len(sizes) < len(_XSIZES) else 4
                sizes.append(min(nb, n_blk - sum(sizes)))
            xdma = []
            b0 = 0
            for di, nb in enumerate(sizes):
                t = xpool.tile([128, 1024 * nb], f8, tag=f"xq_{di}",
                               name=f"xq{b0}", bufs=1)
                xdma.append((b0, nb, t))
                b0 += nb

            def xissue(di):
                bb, nb, t = xdma[di]
                nc.sync.dma_start(out=t,
                                  in_=xwin[:, bb * 1024:(bb + nb) * 1024])

            # first block split by partition halves: the first conv1 pair
            # only reads partitions 0-63, so it can start on the half-DMA
            # group 0 (tiles 0-2, bases 0/32/64, K2<=26) only reads
            # partitions 0-89: split the first DMA so it starts sooner
            t0_ = xdma[0][2]
            nc.sync.dma_start(out=t0_[0:96, :], in_=xwin[0:96, 0:1024])
            nc.sync.dma_start(out=t0_[96:128, :], in_=xwin[96:128, 0:1024])
            xissue(1)
            wcpack_sb = cpool.tile([128, n_wc], f8, tag="wcp",
                                   name="wcpack_sb")
            nc.sync.dma_start(out=wcpack_sb[:, 0:n_wc // 2],
                              in_=wcpack_d[:, 0:n_wc // 2])
            nc.sync.dma_start(out=wcpack_sb[:, n_wc // 2:n_wc],
                              in_=wcpack_d[:, n_wc // 2:n_wc])
            fw2t_sb = cpool.tile_from(fw2t_d[:], name="fw2t_sb")
            for di in range(2, len(sizes)):
                xissue(di)

            def quad_ap(sl, bi):
                blk = sl * n_bps + bi
                for (bb, nb, t) in xdma:
                    if bb <= blk < bb + nb:
                        return t[:, (blk - bb) * 1024:(blk - bb) * 1024 + 1024]
                raise AssertionError(blk)

            # z psum tiles (also the dummy-matmul target for the
            # single-sync-wait preamble)
            zs = [zps.tile([128, _SLICE], f32, tag="z", name=f"z{sl}")
                  for sl in range(_NSL)]

            # single-sync-wait rule: pre-observe PE-read const queues with
            # dummy 1-col matmuls; DVE/ACT-read consts with dummy touches.
            nc.tensor.matmul(zs[0][0:128, 0:1], w1stat_sb[0:26, 0:128],
                             w1stat_sb[0:26, 0:1], start=True, stop=True)
            dvescr = wpool.tile([128, 44], f32, tag="dvescr", name="dvescr",
                                bufs=1)
            nc.vector.tensor_copy(out=dvescr[:, 0:1], in_=biasp_sb[:])
            nc.vector.tensor_copy(out=dvescr[:, 4:44], in_=fb2r_sb[:])
            actscr = wpool.tile([128, 1], f32, tag="actscr", name="actscr",
                                bufs=1)
            nc.scalar.copy(out=actscr[:], in_=bcombN_sb[:])

            recip_insts = []
            for sl in range(_NSL):
                stiles = {}      # group -> s tile
                zu_next = 0
                for gi in range(n_grp):
                    gtiles = list(range(gi * _GRP,
                                        min((gi + 1) * _GRP, n_tiles)))
                    ng_t = len(gtiles)
                    cp = cps.tile([128, _GRP * _SLICE], f32, tag="cp",
                                  name=f"cp{sl}_{gi}")
                    for j, ti in enumerate(gtiles):
                        t = tiles[ti]
                        K2 = t["K2"]
                        bi, g = divmod(ti, 4)
                        quad = quad_ap(sl, bi)
                        rhs = quad[32 * g:32 * g + K2, :].rearrange(
                            "p (two n) -> p two n", two=2)
                        lhsT = w1stat_sb[
                            32 * g:32 * g + K2,
                            bi * 256:bi * 256 + 256].rearrange(
                            "p (two m) -> p two m", two=2)
                        nc.tensor.matmul(
                            cp[:, j * _SLICE:(j + 1) * _SLICE], lhsT,
                            rhs, start=True, stop=True, perf_mode=DR,
                            tile_position=(32 * g, 0))
                    q = qpool.tile([128, _GRP * _SLICE], f32, tag="q",
                                   name=f"q{sl}_{gi}")
                    s = spool.tile([128, _GRP * _SLICE], f8, tag="s",
                                   name=f"s{sl}_{gi}")
                    stiles[gi] = s
                    nc.vector._custom_dve(
                        den_op, out=q[:, 0:ng_t * _SLICE],
                        in0=cp[:, 0:ng_t * _SLICE],
                        s0=biasp_sb[0:128, 0:1], s1=1.0 / 16.0, imm2=2.0)
                    if sl == 0 and gi == 0:
                        # deferred const-queue dummies (single-sync-wait):
                        # emitted after group 0's conv1 so they park in the
                        # PE wait queue while their (later) DMAs land.
                        nc.tensor.matmul(zs[0][0:128, 0:1],
                                         wcpack_sb[0:128, 0:128],
                                         wcpack_sb[0:128, 0:1],
                                         start=True, stop=True)
                        nc.tensor.matmul(zs[0][0:10, 0:1],
                                         fw2t_sb[0:128, 0:10],
                                         fw2t_sb[0:128, 0:1],
                                         start=True, stop=True)
                    if sl == _NSL - 1 and gi == n_grp - 1:
                        # final group's reciprocal on the (otherwise idle)
                        # DVE so the ACT queue can start the table switch
                        # and tail while this group finishes
                        nc.vector._custom_dve(
                            RECIPROCAL_APPROX_FAST,
                            out=s[:, 0:ng_t * _SLICE],
                            in0=q[:, 0:ng_t * _SLICE],
                            s0=RC["s0"], s1=RC["s1"], imm2=RC["imm2"])
                    else:
                        ri = _act_raw(nc, s[:, 0:ng_t * _SLICE],
                                      q[:, 0:ng_t * _SLICE], AF.Reciprocal)
                        recip_insts.append(ri)
                    # z units whose tiles are all covered by groups <= gi
                    while zu_next < len(zunits):
                        kind, t0, col0 = zunits[zu_next]
                        t_last = t0 + (1 if kind == "dr" else 0)
                        if t_last > gtiles[-1]:
                            break
                        sg = stiles[t0 // _GRP]
                        o0 = (t0 % _GRP) * _SLICE
                        if kind == "dr":
                            lhsT = wcpack_sb[
                                :, col0:col0 + 256].rearrange(
                                "p (two m) -> p two m", two=2)
                            rhs = sg[:, o0:o0 + 1024].rearrange(
                                "p (two n) -> p two n", two=2)
                            nc.tensor.matmul(zs[sl], lhsT, rhs,
                                             start=(t0 == 0), stop=False,
                                             perf_mode=DR)
                        else:
                            lhsT = wcpack_sb[:, col0:col0 + 128]
                            rhs = sg[:, o0:o0 + _SLICE]
                            nc.tensor.matmul(
                                zs[sl], lhsT, rhs, start=(t0 == 0),
                                stop=(t_last == n_tiles - 1))
                        zu_next += 1

            # ---- tail: sigmoid via Exp + fast-reciprocal, fc2, log_softmax.
            # (no max-sub: |logits| < 12, exp cannot overflow fp32.)
            # NOTE: must stay after ALL recips - interleaving tail ACT ops
            # with recips thrashes the ACT table sets (measured 8 loads).
            last_recip = recip_insts[-1]
            for sl in range(_NSL):
                e = wpool.tile([128, _SLICE], f32, tag="e", name=f"e{sl}")
                ei = nc.scalar.activation(e, zs[sl], AF.Exp, bias=bcombN_sb[:],
                                          scale=-1.0 / 64.0)
                add_dep_helper(ei.ins, last_recip.ins, sync=False,
                               reason="keep tail ACT after recips (table sets)")
                t1 = wpool.tile([128, _SLICE], f32, tag="t1", name=f"t1{sl}")
                nc.vector.tensor_scalar_add(t1, e, 1.0)
                h = wpool.tile([128, _SLICE], f16, tag="h", name=f"h{sl}")
                nc.vector._custom_dve(RECIPROCAL_APPROX_FAST, out=h, in0=t1,
                                      s0=RC["s0"], s1=RC["s1"], imm2=RC["imm2"])
                ng = _SLICE // 128
                # fc2 psum borrows the cps pool (all 8 banks are spoken for;
                # the cp rotation is idle by the time the tail runs)
                fp = cps.tile([128, _GRP * _SLICE], f32, tag="cp",
                              name=f"fp{sl}")
                for g in range(ng):
                    nc.tensor.matmul(fp[:, g * 10:(g + 1) * 10],
                                     h[:, g * 128:(g + 1) * 128], fw2t_sb[:],
                                     start=True, stop=True)
                lg = wpool.tile([128, 10 * ng], f32, tag="lg", name=f"lg{sl}")
                nc.vector.tensor_tensor(out=lg, in0=fp[:, 0:10 * ng],
                                        in1=fb2r_sb[:, 0:10 * ng],
                                        op=AluOpType.add)
                e2 = wpool.tile([128, 10 * ng], f32, tag="e2", name=f"e2{sl}")
                e2i = nc.scalar.activation(e2, lg, AF.Exp)
                add_dep_helper(e2i.ins, last_recip.ins, sync=False,
                               reason="keep tail ACT after recips (table sets)")
                ssum = wpool.tile([128, ng], f32, tag="ss", name=f"ss{sl}")
                nc.vector.tensor_reduce(
                    ssum, e2.rearrange("p (g k) -> p g k", k=10),
                    axis=mybir.AxisListType.X, op=AluOpType.add)
                lns = wpool.tile([128, ng], f32, tag="ls", name=f"ls{sl}")
                li = nc.scalar.activation(lns, ssum, AF.Ln)
                add_dep_helper(li.ins, last_recip.ins, sync=False,
                               reason="keep tail ACT after recips (table sets)")
                ot = wpool.tile([128, 10 * ng], f32, tag="ot", name=f"ot{sl}")
                nc.vector.tensor_tensor(
                    out=ot.rearrange("p (g k) -> p g k", k=10),
                    in0=lg.rearrange("p (g k) -> p g k", k=10),
                    in1=lns.rearrange("p (g o) -> p g o", o=1).to_broadcast(
                        [128, ng, 10]),
                    op=AluOpType.subtract)
                orow = sl * _SLICE
                oeng = nc.sync if sl == 0 else nc.scalar
                oeng.dma_start(
                    out=out_d[orow:orow + _SLICE, :].rearrange(
                        "(g p) k -> p g k", p=128),
                    in_=ot.rearrange("p (g k) -> p g k", k=10))
    nc.compile()
    return nc


_PROGRAM_CACHE = {}


def kernel(x, w1, b1, w2, b2, fw1, fb1, fw2, fb2):
    global LAST_RESULTS
    tile_wins, consts, tiles = _host_prep(x, w1, b1, w2, b2, fw1, fb1, fw2, fb2)

    if "nc" not in _PROGRAM_CACHE:
        _PROGRAM_CACHE["nc"] = _build_program(tiles)
    nc = _PROGRAM_CACHE["nc"]

    f8 = ml_dtypes.float8_e4m3fn
    blocks = _blocks(tiles)
    n_bps = len(blocks)
    shared = {k: consts[k] for k in ("w1stat", "wcpack", "cb32", "fw2t")}
    in_maps = []
    for c in range(_NCORES):
        m = dict(shared)
        blob = np.zeros((128, _NSL * n_bps * 1024), f8)
        for sl in range(_NSL):
            for bi, blk in enumerate(blocks):
                col0 = (sl * n_bps + bi) * 1024
                for g, t_i in enumerate(blk):
                    t = tiles[t_i]
                    K2 = t["K2"]
                    w = tile_wins[t_i]            # [2, K2, B]
                    c0 = c * _PC + sl * _SLICE
                    for j in range(2):
                        blob[32 * g:32 * g + K2,
                             col0 + j * _SLICE:col0 + (j + 1) * _SLICE] = \
                            w[j, :, c0:c0 + _SLICE]
        m["xwin"] = blob
        in_maps.append(m)

    from concourse.bass_utils import run_bass_kernel_spmd
    trace = bool(int(os.environ.get("BASS_KERNEL_TRACE", "0")))
    res = run_bass_kernel_spmd(nc, in_maps, core_ids=list(range(_NCORES)),
                               trace=trace)
    LAST_RESULTS = res
    return np.concatenate([r["out"] for r in res.results], axis=0)


# revision 70
# speedup vs baseline: 1.0294x; 1.0018x over previous
"""Trainium2 Bass kernel for nn_Net_39230231281866 (dense_cnn).

Network: conv3x3(1->6) -> Taylor-sigmoid -> conv3x3(6->7) -> flatten
         -> fc(4032->128) -> sigmoid -> fc(128->10) -> log_softmax,
batch 8192, data-parallel over 8 NeuronCores (1024 samples/core).

Mapping (v3, fp8 DoubleRow + dense 33-tile conv1 packing):
  * conv2+fc1 folded on the host into one dense GEMM W_comb [128, 4056].
  * conv1 = banded-weight matmul over 33 irregular output tiles (M = 126/
    120/60 rows each, vs 36 rectangular tiles before): the DVE Taylor pass
    is the kernel's pacer and its cost is n_tiles * batch columns, so
    denser M-packing converts directly to wall-clock.
    Tiles: 13 two-row bands x 2 main tiles (21 positions, window 4x13,
    K=52) + 7 leftover tiles (bands' last 5 columns, merged in pairs).
  * Both conv1 and the W_comb GEMM run in fp8e4m3 with
    MatmulPerfMode.DoubleRow (K split into 2 interleaved k-tiles packed
    along the free dim). PSUM accumulation stays fp32. End-to-end
    quantization error ~4e-4 rel (gate is 2e-2).
  * conv1 weights scaled by -8 (= 16 * -1/2); the Taylor denominator
    custom DVE op folds the 1/16 back in: with u = (psum - 8*b1)/16,
    den16(u) = (u(u+1)+1)^2 + u + 2 = u^4+2u^3+3u^2+3u+3  (8/8 v3 stages).
    All tiles start channel-aligned (M % 6 == 0) so one bias vector
    b1[m % 6] serves every tile.
  * All reciprocal ops run on the ScalarE (reciprocal_and_small table),
    writing s as fp8e4 directly; W_comb is scaled by 96 = 64*1.5 (64 keeps
    fp8 weights out of denormals, 1.5 is the den16 normalization), undone
    by the scale of the tail's Exp.
  * Tail sigmoid = 1/(1+exp(-z)) via ACT Exp + DVE reciprocal_approx_fast,
    so the whole tail only needs the natural_log_exp_and_others table:
    exactly 2 ACT table loads per kernel.
  * Input windows are host-packed into per-block (4 tiles at partition
    bases 0/32/64/96) dense [128, 1024] fp8 images; DMA sizes graduate
    (small first blocks) so compute starts ~11us in while the rest
    streams. Start-critical DMAs issue from three different engine queues.
"""

import os
import numpy as np
import ml_dtypes

_B = 8192
_NCORES = 8
_PC = _B // _NCORES
_SLICE = 512
_NSL = _PC // _SLICE

LAST_RESULTS = None

# xwin DMA granularity in blocks (consumption order): small first blocks
# so compute starts early, large later blocks to bound instruction count
_XSIZES = [1, 1, 1, 1, 2, 2, 2, 4, 4]


def _tiles():
    """33 conv1 output tiles, each a channel-aligned run of output
    positions with a compact input window (K <= 64 so half-K <= 32 and
    4 tiles pack into 128 partitions at bases 0/32/64/96)."""
    def mk(pos):
        oys = [p[0] for p in pos]; oxs = [p[1] for p in pos]
        ry0, rx0 = min(oys), min(oxs)
        wr, wc = max(oys) - ry0 + 3, max(oxs) - rx0 + 3
        K = wr * wc
        K2 = (K + 1) // 2
        assert K2 <= 32, (wr, wc)
        return dict(pos=pos, ry0=ry0, rx0=rx0, wr=wr, wc=wc,
                    K=K, K2=K2, M=6 * len(pos))

    mains, leftovers = [], []
    for b in range(13):
        r0 = 2 * b
        pos = [(r0 + rr, c) for c in range(26) for rr in range(2)]
        mains.append(mk(pos[0:21]))
        mains.append(mk(pos[21:42]))
        leftovers.append(pos[42:52])
    ts = list(mains)
    for i in range(6):
        ts.append(mk(leftovers[2 * i] + leftovers[2 * i + 1]))
    ts.append(mk(leftovers[12]))
    assert len(ts) == 33 and sum(t["M"] for t in ts) == 4056
    return ts


# per-slice partition blocks of tiles (4 per block at bases 0/32/64/96)
def _blocks(tiles):
    out, i = [], 0
    while i < len(tiles):
        out.append(list(range(i, min(i + 4, len(tiles)))))
        i += 4
    return out


_GRP = 3  # tiles per psum / den16 / recip / s group


def _z_units(n_tiles):
    """z-GEMM units: ("dr", first_tile, wcpack_col) DoubleRow pairs when
    both tiles share one s group, else ("plain", tile, col) chunks."""
    units, col, t = [], 0, 0
    while t < n_tiles:
        if t + 1 < n_tiles and (t // _GRP) == ((t + 1) // _GRP):
            units.append(("dr", t, col)); col += 256; t += 2
        else:
            units.append(("plain", t, col)); col += 128; t += 1
    return units


def _q8(a):
    return np.asarray(a, np.float32).astype(ml_dtypes.float8_e4m3fn)


def _host_prep(x, w1, b1, w2, b2, fw1, fb1, fw2, fb2):
    x = np.asarray(x, np.float32)
    w1 = np.asarray(w1, np.float32); b1 = np.asarray(b1, np.float32)
    w2 = np.asarray(w2, np.float32); b2 = np.asarray(b2, np.float32)
    fw1 = np.asarray(fw1, np.float32); fb1 = np.asarray(fb1, np.float32)
    fw2 = np.asarray(fw2, np.float32); fb2 = np.asarray(fb2, np.float32)

    tiles = _tiles()
    blocks = _blocks(tiles)
    f8 = ml_dtypes.float8_e4m3fn

    # banded conv1 weights scaled by -8 (psum = -8*conv(x)), DoubleRow
    # interleave, one [K2, 2, 128] stationary per tile at its partition
    # base: w1stat[32g + r, bi*256 + j*128 + m] = wp_t[j*K2 + r, m]
    w1stat = np.zeros((128, 256 * len(blocks)), np.float32)
    for bi, blk in enumerate(blocks):
        for g, t_i in enumerate(blk):
            t = tiles[t_i]
            K2 = t["K2"]
            wp = np.zeros((2 * K2, 128), np.float32)
            for pi_, (oy, ox) in enumerate(t["pos"]):
                for ch in range(6):
                    m = pi_ * 6 + ch
                    for k in range(t["K"]):
                        iy = t["ry0"] + k // t["wc"]
                        ix = t["rx0"] + k % t["wc"]
                        dy, dx = iy - oy, ix - ox
                        if 0 <= dy < 3 and 0 <= dx < 3:
                            wp[k, m] = -8.0 * w1[ch, 0, dy, dx]
            for j in range(2):
                w1stat[32 * g:32 * g + K2,
                       bi * 256 + j * 128:bi * 256 + j * 128 + 128] = \
                    wp[j * K2:(j + 1) * K2, :]

    # fold conv2 + fc1 -> W_comb [128, 6*26*26] (x96 = 64*1.5), b_comb
    fw1r = fw1.reshape(128, 7, 24, 24)
    Wc = np.zeros((128, 6, 26, 26), np.float32)
    for dy in range(3):
        for dx in range(3):
            Wc[:, :, dy:dy + 24, dx:dx + 24] += np.einsum(
                "joyx,oi->jiyx", fw1r, w2[:, :, dy, dx], optimize=True)
    b_comb = fb1 + np.einsum("joyx,o->j", fw1r, b2)
    Wc_flat = (96.0 * Wc.reshape(128, 6 * 26 * 26)).astype(np.float32)

    # W_comb columns. z-units: DoubleRow for tile pairs that live in one
    # 3-tile psum/s group, plain chunks otherwise (see _z_units).
    units = _z_units(len(tiles))
    wcpack = np.zeros((128, 128 * len(tiles)), np.float32)
    for (kind, t0, col0) in units:
        for j in range(2 if kind == "dr" else 1):
            t = tiles[t0 + j]
            # m ordering within tile: pos-major, channel fastest
            rows = [(ch * 26 + oy) * 26 + ox
                    for (oy, ox) in t["pos"] for ch in range(6)]
            wcpack[:t["M"], col0 + j * 128:col0 + (j + 1) * 128] = \
                Wc_flat[:, rows].T

    # f32 const blob: [bias1 | bcombN | fb2r] ([128, 42])
    bias1 = np.array([-8.0 * b1[m % 6] for m in range(128)],
                     np.float32).reshape(128, 1)
    cb32 = np.concatenate(
        [bias1, (-b_comb).reshape(128, 1).astype(np.float32),
         np.tile(fb2.reshape(1, 10), (128, 4)).astype(np.float32)], axis=1)

    consts = dict(
        w1stat=_q8(w1stat), wcpack=_q8(wcpack),
        cb32=np.ascontiguousarray(cb32),
        fw2t=np.ascontiguousarray(fw2.T).astype(np.float16),   # [128, 10]
    )

    # per-tile window data [2, K2, B] fp8 (k-tile-interleaved, zero-padded)
    x_pm = _q8(x.reshape(_B, 784).T)                           # [784, B]
    tile_wins = []
    for t in tiles:
        K2 = t["K2"]
        rows = ((np.arange(t["wr"])[:, None] + t["ry0"]) * 28 +
                (np.arange(t["wc"])[None, :] + t["rx0"])).reshape(-1)
        w = np.zeros((2 * K2, _B), f8)
        w[:t["K"], :] = x_pm[rows, :]
        tile_wins.append(w.reshape(2, K2, _B))
    return tile_wins, consts, tiles


def _register_taylor_den16s():
    import concourse.dve_ops as dve_ops
    name = "TAYLOR_DEN16S_ANT"
    if name in dve_ops._SUB_OPCODE_FOR_NAME:
        return next(o for o in dve_ops.OPS if o.name == name)
    from concourse.dve_spec import Spec, Src0, C0, C1, C2, One, lower, sq
    from concourse.dve_uop import DveOpSpec

    # u = (in0 + s0) * s1;  out = (u*(u+1)+1)^2 + u + imm2
    #   == u^4 + 2u^3 + 3u^2 + 3u + (1 + imm2)   (imm2 = 2 -> den16)
    u = (Src0 + C0) * C1
    body = sq(u * (u + One) + One) + u + C2

    def _ref(in0, in1, s0, s1, imm2):
        uu = (in0.astype(np.float32) + s0) * s1
        return (uu * (uu + 1.0) + 1.0) ** 2 + uu + imm2

    spec = Spec(body=body, reference=_ref)
    row = max(dve_ops._SUB_OPCODE_FOR_NAME.values()) + 1
    assert row < 0x20
    shas = {ver: DveOpSpec(name=name, opcode=row, uops=lower(spec, ver=ver),
                           rd1_en=False).sha(ver)
            for ver in ("v3", "v4")}
    op = dve_ops.DveOp(name, spec, subdim=False, uops_sha=shas)
    dve_ops.OPS.append(op)
    dve_ops.CUSTOM_DVE_SPECS[op.name] = op.spec
    dve_ops._SUB_OPCODE_FOR_NAME[op.name] = row
    return op


def _pin_act_tables():
    """Pin Copy -> reciprocal_and_small and Exp/Ln ->
    natural_log_exp_and_others so the kernel loads exactly 2 ACT tables."""
    import concourse.bacc as bacc
    import concourse.mybir as mybir
    if getattr(bacc, "_ant_tables_pinned", False):
        return
    orig = bacc.get_activation_tables
    AF = mybir.ActivationFunctionType

    def patched(arch):
        tabs = {k: set(v) for k, v in orig(arch).items()}
        for name, fns in tabs.items():
            if name != "natural_log_exp_and_others":
                fns.discard(AF.Exp)
                fns.discard(AF.Ln)
            if name != "reciprocal_and_small":
                fns.discard(AF.Copy)
        return tabs

    bacc.get_activation_tables = patched
    bacc._ant_tables_pinned = True


def _act_raw(nc, out, in_, func, bias=0.0, scale=1.0):
    """Emit InstActivation directly (used for Reciprocal, which the
    nc.scalar.activation wrapper refuses; measured ~1.2e-5 rel err)."""
    import concourse.mybir as mybir
    eng = nc.scalar
    inputs = [eng.lower_ap(in_)]
    for arg in (bias, scale, 0.0):
        inputs.append(mybir.ImmediateValue(dtype=mybir.dt.float32,
                                           value=float(arg)))
    return eng.add_instruction(mybir.InstActivation(
        name=nc.get_next_instruction_name(), func=func, ins=inputs,
        outs=[eng.lower_ap(out)]))


def _build_program(tiles):
    import concourse.bacc as bacc
    import concourse.mybir as mybir
    from concourse.tile import TileContext
    from concourse.tile_rust import add_dep_helper
    from concourse.alu_op_type import AluOpType
    from concourse.dve_ops import RECIP_APPROX_FAST_CONSTS as RC
    from concourse.dve_ops import RECIPROCAL_APPROX_FAST

    f32 = mybir.dt.float32
    f16 = mybir.dt.float16
    f8 = mybir.dt.float8e4
    AF = mybir.ActivationFunctionType
    DR = mybir.MatmulPerfMode.DoubleRow
    den_op = _register_taylor_den16s()
    _pin_act_tables()

    nc = bacc.Bacc()
    blocks = _blocks(tiles)
    n_bps = len(blocks)                      # blocks per slice (9)
    n_tiles = len(tiles)
    zunits = _z_units(n_tiles)
    n_grp = (n_tiles + _GRP - 1) // _GRP     # psum/s groups per slice (11)
    xwin = nc.declare_dram_parameter("xwin", [128, _NSL * n_bps * 1024], f8,
                                     isOutput=False)
    w1stat_d = nc.declare_dram_parameter("w1stat", [128, 256 * n_bps], f8,
                                         isOutput=False)
    wcpack_d = nc.declare_dram_parameter("wcpack", [128, 128 * n_tiles], f8,
                                         isOutput=False)
    cb32_d = nc.declare_dram_parameter("cb32", [128, 42], f32, isOutput=False)
    fw2t_d = nc.declare_dram_parameter("fw2t", [128, 10], f16, isOutput=False)
    out_d = nc.declare_dram_parameter("out", [_PC, 10], f32, isOutput=True)
    n_wc = 128 * n_tiles

    with TileContext(nc) as tc:
        with (
            tc.tile_pool(name="const", bufs=1) as cpool,
            tc.tile_pool(name="xq", bufs=5) as xpool,
            tc.tile_pool(name="q", bufs=4) as qpool,
            tc.tile_pool(name="s", bufs=4) as spool,
            tc.tile_pool(name="work", bufs=3) as wpool,
            tc.tile_pool(name="cps", bufs=2, space="PSUM") as cps,
            tc.tile_pool(name="zps", bufs=2, space="PSUM") as zps,
        ):
            # DMA order tuned for pipeline start: tiny cb32 (DVE preamble
            # reads it) and w1stat first (on side queues), first input
            # blocks, then wcpack split in two for parallel transfer (first
            # z matmul needs it ~15us in), then the rest of the inputs.
            POOLE = mybir.EngineType.Pool
            ACTE = mybir.EngineType.Activation
            cb32_sb = cpool.tile_from(cb32_d[:], name="cb32_sb",
                                      forced_dma_engine=POOLE)
            biasp_sb = cb32_sb[:, 0:1]
            bcombN_sb = cb32_sb[:, 1:2]
            fb2r_sb = cb32_sb[:, 2:42]
            # w1stat split across two queues (295KB on one queue would gate
            # the first conv1 by ~6us); block 0's columns go first so the
            # first conv1 is gated only by the first input half-DMA
            w1stat_sb = cpool.tile([128, 256 * n_bps], f8, tag="w1s",
                                   name="w1stat_sb")
            h1 = 256 * (n_bps // 2)
            nc.scalar.dma_start(out=w1stat_sb[:, 0:256],
                                in_=w1stat_d[:, 0:256])
            nc.scalar.dma_start(out=w1stat_sb[:, 256:h1],
                                in_=w1stat_d[:, 256:h1])
            nc.gpsimd.dma_start(out=w1stat_sb[:, h1:256 * n_bps],
                                in_=w1stat_d[:, h1:256 * n_bps])

            n_blk = _NSL * n_bps
            sizes = []
            while sum(sizes) < n_blk:
                nb = _XSIZES[len(sizes)] if len(sizes) < len(_XSIZES) else 4
                sizes.append(min(nb, n_blk - sum(sizes)))
            xdma = []
            b0 = 0
            for di, nb in enumerate(sizes):
                t = xpool.tile([128, 1024 * nb], f8, tag=f"xq_{di}",
                               name=f"xq{b0}", bufs=1)
                xdma.append((b0, nb, t))
                b0 += nb

            def xissue(di):
                bb, nb, t = xdma[di]
                nc.sync.dma_start(out=t,
                                  in_=xwin[:, bb * 1024:(bb + nb) * 1024])

            # first block split by partition halves: the first conv1 pair
            # only reads partitions 0-63, so it can start on the half-DMA
            # group 0 (tiles 0-2, bases 0/32/64, K2<=26) only reads
            # partitions 0-89: split the first DMA so it starts sooner
            t0_ = xdma[0][2]
            nc.sync.dma_start(out=t0_[0:96, :], in_=xwin[0:96, 0:1024])
            nc.sync.dma_start(out=t0_[96:128, :], in_=xwin[96:128, 0:1024])
            xissue(1)
            wcpack_sb = cpool.tile([128, n_wc], f8, tag="wcp",
                                   name="wcpack_sb")
            nc.sync.dma_start(out=wcpack_sb[:, 0:n_wc // 2],
                              in_=wcpack_d[:, 0:n_wc // 2])
            nc.sync.dma_start(out=wcpack_sb[:, n_wc // 2:n_wc],
                              in_=wcpack_d[:, n_wc // 2:n_wc])
            fw2t_sb = cpool.tile_from(fw2t_d[:], name="fw2t_sb")
            for di in range(2, len(sizes)):
                xissue(di)

            def quad_ap(sl, bi):
                blk = sl * n_bps + bi
                for (bb, nb, t) in xdma:
                    if bb <= blk < bb + nb:
                        return t[:, (blk - bb) * 1024:(blk - bb) * 1024 + 1024]
                raise AssertionError(blk)

            # z psum tiles (also the dummy-matmul target for the
            # single-sync-wait preamble)
            zs = [zps.tile([128, _SLICE], f32, tag="z", name=f"z{sl}")
                  for sl in range(_NSL)]

            # single-sync-wait rule: pre-observe PE-read const queues with
            # dummy 1-col matmuls; DVE/ACT-read consts with dummy touches.
            nc.tensor.matmul(zs[0][0:128, 0:1], w1stat_sb[0:26, 0:128],
                             w1stat_sb[0:26, 0:1], start=True, stop=True)
            dvescr = wpool.tile([128, 44], f32, tag="dvescr", name="dvescr",
                                bufs=1)
            nc.vector.tensor_copy(out=dvescr[:, 0:1], in_=biasp_sb[:])
            nc.vector.tensor_copy(out=dvescr[:, 4:44], in_=fb2r_sb[:])
            actscr = wpool.tile([128, 1], f32, tag="actscr", name="actscr",
                                bufs=1)
            nc.scalar.copy(out=actscr[:], in_=bcombN_sb[:])

            recip_insts = []
            for sl in range(_NSL):
                stiles = {}      # group -> s tile
                zu_next = 0
                for gi in range(n_grp):
                    gtiles = list(range(gi * _GRP,
                                        min((gi + 1) * _GRP, n_tiles)))
                    ng_t = len(gtiles)
                    cp = cps.tile([128, _GRP * _SLICE], f32, tag="cp",
                                  name=f"cp{sl}_{gi}")
                    for j, ti in enumerate(gtiles):
                        t = tiles[ti]
                        K2 = t["K2"]
                        bi, g = divmod(ti, 4)
                        quad = quad_ap(sl, bi)
                        rhs = quad[32 * g:32 * g + K2, :].rearrange(
                            "p (two n) -> p two n", two=2)
                        lhsT = w1stat_sb[
                            32 * g:32 * g + K2,
                            bi * 256:bi * 256 + 256].rearrange(
                            "p (two m) -> p two m", two=2)
                        nc.tensor.matmul(
                            cp[:, j * _SLICE:(j + 1) * _SLICE], lhsT,
                            rhs, start=True, stop=True, perf_mode=DR,
                            tile_position=(32 * g, 0))
                    q = qpool.tile([128, _GRP * _SLICE], f32, tag="q",
                                   name=f"q{sl}_{gi}")
                    s = spool.tile([128, _GRP * _SLICE], f8, tag="s",
                                   name=f"s{sl}_{gi}")
                    stiles[gi] = s
                    nc.vector._custom_dve(
                        den_op, out=q[:, 0:ng_t * _SLICE],
                        in0=cp[:, 0:ng_t * _SLICE],
                        s0=biasp_sb[0:128, 0:1], s1=1.0 / 16.0, imm2=2.0)
                    if sl == 0 and gi == 0:
                        # deferred const-queue dummies (single-sync-wait):
                        # emitted after group 0's conv1 so they park in the
                        # PE wait queue while their (later) DMAs land.
                        nc.tensor.matmul(zs[0][0:128, 0:1],
                                         wcpack_sb[0:128, 0:128],
                                         wcpack_sb[0:128, 0:1],
                                         start=True, stop=True)
                        nc.tensor.matmul(zs[0][0:10, 0:1],
                                         fw2t_sb[0:128, 0:10],
                                         fw2t_sb[0:128, 0:1],
                                         start=True, stop=True)
                    if sl == _NSL - 1 and gi == n_grp - 1:
                        # final group's reciprocal on the (otherwise idle)
                        # DVE so the ACT queue can start the table switch
                        # and tail while this group finishes
                        nc.vector._custom_dve(
                            RECIPROCAL_APPROX_FAST,
                            out=s[:, 0:ng_t * _SLICE],
                            in0=q[:, 0:ng_t * _SLICE],
                            s0=RC["s0"], s1=RC["s1"], imm2=RC["imm2"])
                    else:
                        ri = _act_raw(nc, s[:, 0:ng_t * _SLICE],
                                      q[:, 0:ng_t * _SLICE], AF.Reciprocal)
                        recip_insts.append(ri)
                    # z units whose tiles are all covered by groups <= gi
                    while zu_next < len(zunits):
                        kind, t0, col0 = zunits[zu_next]
                        t_last = t0 + (1 if kind == "dr" else 0)
                        if t_last > gtiles[-1]:
                            break
                        sg = stiles[t0 // _GRP]
                        o0 = (t0 % _GRP) * _SLICE
                        if kind == "dr":
                            lhsT = wcpack_sb[
                                :, col0:col0 + 256].rearrange(
                                "p (two m) -> p two m", two=2)
                            rhs = sg[:, o0:o0 + 1024].rearrange(
                                "p (two n) -> p two n", two=2)
                            nc.tensor.matmul(zs[sl], lhsT, rhs,
                                             start=(t0 == 0), stop=False,
                                             perf_mode=DR)
                        else:
                            lhsT = wcpack_sb[:, col0:col0 + 128]
                            rhs = sg[:, o0:o0 + _SLICE]
                            nc.tensor.matmul(
                                zs[sl], lhsT, rhs, start=(t0 == 0),
                                stop=(t_last == n_tiles - 1))
                        zu_next += 1

            # ---- tail: sigmoid via Exp + fast-reciprocal, fc2, log_softmax.
            # (no max-sub: |logits| < 12, exp cannot overflow fp32.)
            # NOTE: must stay after ALL recips - interleaving tail ACT ops
            # with recips thrashes the ACT table sets (measured 8 loads).
            last_recip = recip_insts[-1]
            for sl in range(_NSL):
                e = wpool.tile([128, _SLICE], f32, tag="e", name=f"e{sl}")
                ei = nc.scalar.activation(e, zs[sl], AF.Exp, bias=bcombN_sb[:],
                                          scale=-1.0 / 64.0)
                add_dep_helper(ei.ins, last_recip.ins, sync=False,
                               reason="keep tail ACT after recips (table sets)")
                t1 = wpool.tile([128, _SLICE], f32, tag="t1", name=f"t1{sl}")
                nc.vector.tensor_scalar_add(t1, e, 1.0)
                h = wpool.tile([128, _SLICE], f16, tag="h", name=f"h{sl}")
                nc.vector._custom_dve(RECIPROCAL_APPROX_FAST, out=h, in0=t1,
                                      s0=RC["s0"], s1=RC["s1"], imm2=RC["imm2"])
                ng = _SLICE // 128
                # fc2 psum borrows the cps pool (all 8 banks are spoken for;
                # the cp rotation is idle by the time the tail runs)
                fp = cps.tile([128, _GRP * _SLICE], f32, tag="cp",
                              name=f"fp{sl}")
                for g in range(ng):
                    nc.tensor.matmul(fp[:, g * 10:(g + 1) * 10],
                                     h[:, g * 128:(g + 1) * 128], fw2t_sb[:],
                                     start=True, stop=True)
                lg = wpool.tile([128, 10 * ng], f32, tag="lg", name=f"lg{sl}")
                nc.vector.tensor_tensor(out=lg, in0=fp[:, 0:10 * ng],
                                        in1=fb2r_sb[:, 0:10 * ng],
                                        op=AluOpType.add)
                e2 = wpool.tile([128, 10 * ng], f32, tag="e2", name=f"e2{sl}")
                e2i = nc.scalar.activation(e2, lg, AF.Exp)
                add_dep_helper(e2i.ins, last_recip.ins, sync=False,
                               reason="keep tail ACT after recips (table sets)")
                ssum = wpool.tile([128, ng], f32, tag="ss", name=f"ss{sl}")
                nc.vector.tensor_reduce(
                    ssum, e2.rearrange("p (g k) -> p g k", k=10),
                    axis=mybir.AxisListType.X, op=AluOpType.add)
                lns = wpool.tile([128, ng], f32, tag="ls", name=f"ls{sl}")
                li = nc.scalar.activation(lns, ssum, AF.Ln)
                add_dep_helper(li.ins, last_recip.ins, sync=False,
                               reason="keep tail ACT after recips (table sets)")
                ot = wpool.tile([128, 10 * ng], f32, tag="ot", name=f"ot{sl}")
                nc.vector.tensor_tensor(
                    out=ot.rearrange("p (g k) -> p g k", k=10),
                    in0=lg.rearrange("p (g k) -> p g k", k=10),
                    in1=lns.rearrange("p (g o) -> p g o", o=1).to_broadcast(
                        [128, ng, 10]),
                    op=AluOpType.subtract)
                orow = sl * _SLICE
                oeng = nc.sync if sl == 0 else nc.scalar
                oeng.dma_start(
                    out=out_d[orow:orow + _SLICE, :].rearrange(
                        "(g p) k -> p g k", p=128),
                    in_=ot.rearrange("p (g k) -> p g k", k=10))
    nc.compile()
    return nc


_PROGRAM_CACHE = {}


def kernel(x, w1, b1, w2, b2, fw1, fb1, fw2, fb2):
    global LAST_RESULTS
    tile_wins, consts, tiles = _host_prep(x, w1, b1, w2, b2, fw1, fb1, fw2, fb2)

    if "nc" not in _PROGRAM_CACHE:
        _PROGRAM_CACHE["nc"] = _build_program(tiles)
    nc = _PROGRAM_CACHE["nc"]

    f8 = ml_dtypes.float8_e4m3fn
    blocks = _blocks(tiles)
    n_bps = len(blocks)
    shared = {k: consts[k] for k in ("w1stat", "wcpack", "cb32", "fw2t")}
    in_maps = []
    for c in range(_NCORES):
        m = dict(shared)
        blob = np.zeros((128, _NSL * n_bps * 1024), f8)
        for sl in range(_NSL):
            for bi, blk in enumerate(blocks):
                col0 = (sl * n_bps + bi) * 1024
                for g, t_i in enumerate(blk):
                    t = tiles[t_i]
                    K2 = t["K2"]
                    w = tile_wins[t_i]            # [2, K2, B]
                    c0 = c * _PC + sl * _SLICE
                    for j in range(2):
                        blob[32 * g:32 * g + K2,
                             col0 + j * _SLICE:col0 + (j + 1) * _SLICE] = \
                            w[j, :, c0:c0 + _SLICE]
        m["xwin"] = blob
        in_maps.append(m)

    from concourse.bass_utils import run_bass_kernel_spmd
    trace = bool(int(os.environ.get("BASS_KERNEL_TRACE", "0")))
    res = run_bass_kernel_spmd(nc, in_maps, core_ids=list(range(_NCORES)),
                               trace=trace)
    LAST_RESULTS = res
    return np.concatenate([r["out"] for r in res.results], axis=0)


# revision 71
# speedup vs baseline: 1.0494x; 1.0195x over previous
"""Trainium2 Bass kernel for nn_Net_39230231281866 (dense_cnn).

Network: conv3x3(1->6) -> Taylor-sigmoid -> conv3x3(6->7) -> flatten
         -> fc(4032->128) -> sigmoid -> fc(128->10) -> log_softmax,
batch 8192, data-parallel over 8 NeuronCores (1024 samples/core).

Mapping (v3, fp8 DoubleRow + dense 33-tile conv1 packing):
  * conv2+fc1 folded on the host into one dense GEMM W_comb [128, 4056].
  * conv1 = banded-weight matmul over 33 irregular output tiles (M = 126/
    120/60 rows each, vs 36 rectangular tiles before): the DVE Taylor pass
    is the kernel's pacer and its cost is n_tiles * batch columns, so
    denser M-packing converts directly to wall-clock.
    Tiles: 13 two-row bands x 2 main tiles (21 positions, window 4x13,
    K=52) + 7 leftover tiles (bands' last 5 columns, merged in pairs).
  * Both conv1 and the W_comb GEMM run in fp8e4m3 with
    MatmulPerfMode.DoubleRow (K split into 2 interleaved k-tiles packed
    along the free dim). PSUM accumulation stays fp32. End-to-end
    quantization error ~4e-4 rel (gate is 2e-2).
  * conv1 weights scaled by -8 (= 16 * -1/2); the Taylor denominator
    custom DVE op folds the 1/16 back in: with u = (psum - 8*b1)/16,
    den16(u) = (u(u+1)+1)^2 + u + 2 = u^4+2u^3+3u^2+3u+3  (8/8 v3 stages).
    All tiles start channel-aligned (M % 6 == 0) so one bias vector
    b1[m % 6] serves every tile.
  * All reciprocal ops run on the ScalarE (reciprocal_and_small table),
    writing s as fp8e4 directly; W_comb is scaled by 96 = 64*1.5 (64 keeps
    fp8 weights out of denormals, 1.5 is the den16 normalization), undone
    by the scale of the tail's Exp.
  * Tail sigmoid = 1/(1+exp(-z)) via ACT Exp + DVE reciprocal_approx_fast,
    so the whole tail only needs the natural_log_exp_and_others table:
    exactly 2 ACT table loads per kernel.
  * Input windows are host-packed into per-block (4 tiles at partition
    bases 0/32/64/96) dense [128, 1024] fp8 images; DMA sizes graduate
    (small first blocks) so compute starts ~11us in while the rest
    streams. Start-critical DMAs issue from three different engine queues.
"""

import os
import numpy as np
import ml_dtypes

_B = 8192
_NCORES = 8
_PC = _B // _NCORES
_SLICE = 512
_NSL = _PC // _SLICE

LAST_RESULTS = None

# xwin DMA granularity in blocks (consumption order): small first blocks
# so compute starts early, large later blocks to bound instruction count
_XSIZES = [1, 1, 1, 1, 2, 2, 2, 4, 4]


def _tiles():
    """33 conv1 output tiles, each a channel-aligned run of output
    positions with a compact input window (K <= 64 so half-K <= 32 and
    4 tiles pack into 128 partitions at bases 0/32/64/96)."""
    def mk(pos):
        oys = [p[0] for p in pos]; oxs = [p[1] for p in pos]
        ry0, rx0 = min(oys), min(oxs)
        wr, wc = max(oys) - ry0 + 3, max(oxs) - rx0 + 3
        K = wr * wc
        K2 = (K + 1) // 2
        assert K2 <= 32, (wr, wc)
        return dict(pos=pos, ry0=ry0, rx0=rx0, wr=wr, wc=wc,
                    K=K, K2=K2, M=6 * len(pos))

    mains, leftovers = [], []
    for b in range(13):
        r0 = 2 * b
        pos = [(r0 + rr, c) for c in range(26) for rr in range(2)]
        mains.append(mk(pos[0:21]))
        mains.append(mk(pos[21:42]))
        leftovers.append(pos[42:52])
    ts = list(mains)
    for i in range(6):
        ts.append(mk(leftovers[2 * i] + leftovers[2 * i + 1]))
    ts.append(mk(leftovers[12]))
    assert len(ts) == 33 and sum(t["M"] for t in ts) == 4056
    return ts


# per-slice partition blocks of tiles (4 per block at bases 0/32/64/96)
def _blocks(tiles):
    out, i = [], 0
    while i < len(tiles):
        out.append(list(range(i, min(i + 4, len(tiles)))))
        i += 4
    return out


_GRP = 3  # tiles per psum / den16 / recip / s group


def _z_units(n_tiles):
    """z-GEMM units: ("dr", first_tile, wcpack_col) DoubleRow pairs when
    both tiles share one s group, else ("plain", tile, col) chunks."""
    units, col, t = [], 0, 0
    while t < n_tiles:
        if t + 1 < n_tiles and (t // _GRP) == ((t + 1) // _GRP):
            units.append(("dr", t, col)); col += 256; t += 2
        else:
            units.append(("plain", t, col)); col += 128; t += 1
    return units


def _q8(a):
    return np.asarray(a, np.float32).astype(ml_dtypes.float8_e4m3fn)


def _host_prep(x, w1, b1, w2, b2, fw1, fb1, fw2, fb2):
    x = np.asarray(x, np.float32)
    w1 = np.asarray(w1, np.float32); b1 = np.asarray(b1, np.float32)
    w2 = np.asarray(w2, np.float32); b2 = np.asarray(b2, np.float32)
    fw1 = np.asarray(fw1, np.float32); fb1 = np.asarray(fb1, np.float32)
    fw2 = np.asarray(fw2, np.float32); fb2 = np.asarray(fb2, np.float32)

    tiles = _tiles()
    blocks = _blocks(tiles)
    f8 = ml_dtypes.float8_e4m3fn

    # banded conv1 weights scaled by -8 (psum = -8*conv(x)), DoubleRow
    # interleave, one [K2, 2, 128] stationary per tile at its partition
    # base: w1stat[32g + r, bi*256 + j*128 + m] = wp_t[j*K2 + r, m]
    w1stat = np.zeros((128, 256 * len(blocks)), np.float32)
    for bi, blk in enumerate(blocks):
        for g, t_i in enumerate(blk):
            t = tiles[t_i]
            K2 = t["K2"]
            wp = np.zeros((2 * K2, 128), np.float32)
            for pi_, (oy, ox) in enumerate(t["pos"]):
                for ch in range(6):
                    m = pi_ * 6 + ch
                    for k in range(t["K"]):
                        iy = t["ry0"] + k // t["wc"]
                        ix = t["rx0"] + k % t["wc"]
                        dy, dx = iy - oy, ix - ox
                        if 0 <= dy < 3 and 0 <= dx < 3:
                            wp[k, m] = -8.0 * w1[ch, 0, dy, dx]
            for j in range(2):
                w1stat[32 * g:32 * g + K2,
                       bi * 256 + j * 128:bi * 256 + j * 128 + 128] = \
                    wp[j * K2:(j + 1) * K2, :]

    # fold conv2 + fc1 -> W_comb [128, 6*26*26] (x96 = 64*1.5), b_comb
    fw1r = fw1.reshape(128, 7, 24, 24)
    Wc = np.zeros((128, 6, 26, 26), np.float32)
    for dy in range(3):
        for dx in range(3):
            Wc[:, :, dy:dy + 24, dx:dx + 24] += np.einsum(
                "joyx,oi->jiyx", fw1r, w2[:, :, dy, dx], optimize=True)
    b_comb = fb1 + np.einsum("joyx,o->j", fw1r, b2)
    Wc_flat = (96.0 * Wc.reshape(128, 6 * 26 * 26)).astype(np.float32)

    # W_comb columns. z-units: DoubleRow for tile pairs that live in one
    # 3-tile psum/s group, plain chunks otherwise (see _z_units).
    units = _z_units(len(tiles))
    wcpack = np.zeros((128, 128 * len(tiles)), np.float32)
    for (kind, t0, col0) in units:
        for j in range(2 if kind == "dr" else 1):
            t = tiles[t0 + j]
            # m ordering within tile: pos-major, channel fastest
            rows = [(ch * 26 + oy) * 26 + ox
                    for (oy, ox) in t["pos"] for ch in range(6)]
            wcpack[:t["M"], col0 + j * 128:col0 + (j + 1) * 128] = \
                Wc_flat[:, rows].T

    # f32 const blob: [bias1 | bcombN | fb2r] ([128, 42])
    bias1 = np.array([-8.0 * b1[m % 6] for m in range(128)],
                     np.float32).reshape(128, 1)
    cb32 = np.concatenate(
        [bias1, (-b_comb).reshape(128, 1).astype(np.float32),
         np.tile(fb2.reshape(1, 10), (128, 4)).astype(np.float32)], axis=1)

    consts = dict(
        w1stat=_q8(w1stat), wcpack=_q8(wcpack),
        cb32=np.ascontiguousarray(cb32),
        fw2t=np.ascontiguousarray(fw2.T).astype(np.float16),   # [128, 10]
    )

    # per-tile window data [2, K2, B] fp8 (k-tile-interleaved, zero-padded)
    x_pm = _q8(x.reshape(_B, 784).T)                           # [784, B]
    tile_wins = []
    for t in tiles:
        K2 = t["K2"]
        rows = ((np.arange(t["wr"])[:, None] + t["ry0"]) * 28 +
                (np.arange(t["wc"])[None, :] + t["rx0"])).reshape(-1)
        w = np.zeros((2 * K2, _B), f8)
        w[:t["K"], :] = x_pm[rows, :]
        tile_wins.append(w.reshape(2, K2, _B))
    return tile_wins, consts, tiles


def _register_taylor_den16s():
    import concourse.dve_ops as dve_ops
    name = "TAYLOR_DEN16S_ANT"
    if name in dve_ops._SUB_OPCODE_FOR_NAME:
        return next(o for o in dve_ops.OPS if o.name == name)
    from concourse.dve_spec import Spec, Src0, C0, C1, C2, One, lower, sq
    from concourse.dve_uop import DveOpSpec

    # u = (in0 + s0) * s1;  out = (u*(u+1)+1)^2 + u + imm2
    #   == u^4 + 2u^3 + 3u^2 + 3u + (1 + imm2)   (imm2 = 2 -> den16)
    u = (Src0 + C0) * C1
    body = sq(u * (u + One) + One) + u + C2

    def _ref(in0, in1, s0, s1, imm2):
        uu = (in0.astype(np.float32) + s0) * s1
        return (uu * (uu + 1.0) + 1.0) ** 2 + uu + imm2

    spec = Spec(body=body, reference=_ref)
    row = max(dve_ops._SUB_OPCODE_FOR_NAME.values()) + 1
    assert row < 0x20
    shas = {ver: DveOpSpec(name=name, opcode=row, uops=lower(spec, ver=ver),
                           rd1_en=False).sha(ver)
            for ver in ("v3", "v4")}
    op = dve_ops.DveOp(name, spec, subdim=False, uops_sha=shas)
    dve_ops.OPS.append(op)
    dve_ops.CUSTOM_DVE_SPECS[op.name] = op.spec
    dve_ops._SUB_OPCODE_FOR_NAME[op.name] = row
    return op


def _pin_act_tables():
    """Pin Copy -> reciprocal_and_small and Exp/Ln ->
    natural_log_exp_and_others so the kernel loads exactly 2 ACT tables."""
    import concourse.bacc as bacc
    import concourse.mybir as mybir
    if getattr(bacc, "_ant_tables_pinned", False):
        return
    orig = bacc.get_activation_tables
    AF = mybir.ActivationFunctionType

    def patched(arch):
        tabs = {k: set(v) for k, v in orig(arch).items()}
        for name, fns in tabs.items():
            if name != "natural_log_exp_and_others":
                fns.discard(AF.Exp)
                fns.discard(AF.Ln)
            if name != "reciprocal_and_small":
                fns.discard(AF.Copy)
        return tabs

    bacc.get_activation_tables = patched
    bacc._ant_tables_pinned = True


def _act_raw(nc, out, in_, func, bias=0.0, scale=1.0):
    """Emit InstActivation directly (used for Reciprocal, which the
    nc.scalar.activation wrapper refuses; measured ~1.2e-5 rel err)."""
    import concourse.mybir as mybir
    eng = nc.scalar
    inputs = [eng.lower_ap(in_)]
    for arg in (bias, scale, 0.0):
        inputs.append(mybir.ImmediateValue(dtype=mybir.dt.float32,
                                           value=float(arg)))
    return eng.add_instruction(mybir.InstActivation(
        name=nc.get_next_instruction_name(), func=func, ins=inputs,
        outs=[eng.lower_ap(out)]))


def _build_program(tiles):
    import concourse.bacc as bacc
    import concourse.mybir as mybir
    from concourse.tile import TileContext
    from concourse.tile_rust import add_dep_helper
    from concourse.alu_op_type import AluOpType
    from concourse.dve_ops import RECIP_APPROX_FAST_CONSTS as RC
    from concourse.dve_ops import RECIPROCAL_APPROX_FAST

    f32 = mybir.dt.float32
    f16 = mybir.dt.float16
    f8 = mybir.dt.float8e4
    AF = mybir.ActivationFunctionType
    DR = mybir.MatmulPerfMode.DoubleRow
    den_op = _register_taylor_den16s()
    _pin_act_tables()

    nc = bacc.Bacc()
    blocks = _blocks(tiles)
    n_bps = len(blocks)                      # blocks per slice (9)
    n_tiles = len(tiles)
    zunits = _z_units(n_tiles)
    n_grp = (n_tiles + _GRP - 1) // _GRP     # psum/s groups per slice (11)
    xwin = nc.declare_dram_parameter("xwin", [128, _NSL * n_bps * 1024], f8,
                                     isOutput=False)
    w1stat_d = nc.declare_dram_parameter("w1stat", [128, 256 * n_bps], f8,
                                         isOutput=False)
    wcpack_d = nc.declare_dram_parameter("wcpack", [128, 128 * n_tiles], f8,
                                         isOutput=False)
    cb32_d = nc.declare_dram_parameter("cb32", [128, 42], f32, isOutput=False)
    fw2t_d = nc.declare_dram_parameter("fw2t", [128, 10], f16, isOutput=False)
    out_d = nc.declare_dram_parameter("out", [_PC, 10], f32, isOutput=True)
    n_wc = 128 * n_tiles

    with TileContext(nc) as tc:
        with (
            tc.tile_pool(name="const", bufs=1) as cpool,
            tc.tile_pool(name="xq", bufs=5) as xpool,
            tc.tile_pool(name="q", bufs=3) as qpool,
            tc.tile_pool(name="s", bufs=3) as spool,
            tc.tile_pool(name="work", bufs=3) as wpool,
            tc.tile_pool(name="cps", bufs=2, space="PSUM") as cps,
            tc.tile_pool(name="zps", bufs=2, space="PSUM") as zps,
        ):
            # DMA order tuned for pipeline start: tiny cb32 (DVE preamble
            # reads it) and w1stat first (on side queues), first input
            # blocks, then wcpack split in two for parallel transfer (first
            # z matmul needs it ~15us in), then the rest of the inputs.
            POOLE = mybir.EngineType.Pool
            ACTE = mybir.EngineType.Activation
            cb32_sb = cpool.tile_from(cb32_d[:], name="cb32_sb",
                                      forced_dma_engine=POOLE)
            biasp_sb = cb32_sb[:, 0:1]
            bcombN_sb = cb32_sb[:, 1:2]
            fb2r_sb = cb32_sb[:, 2:42]
            # w1stat split across two queues (295KB on one queue would gate
            # the first conv1 by ~6us); block 0's columns go first so the
            # first conv1 is gated only by the first input half-DMA
            w1stat_sb = cpool.tile([128, 256 * n_bps], f8, tag="w1s",
                                   name="w1stat_sb")
            h1 = 256 * (n_bps // 2)
            nc.scalar.dma_start(out=w1stat_sb[:, 0:256],
                                in_=w1stat_d[:, 0:256])
            nc.scalar.dma_start(out=w1stat_sb[:, 256:h1],
                                in_=w1stat_d[:, 256:h1])
            nc.gpsimd.dma_start(out=w1stat_sb[:, h1:256 * n_bps],
                                in_=w1stat_d[:, h1:256 * n_bps])

            n_blk = _NSL * n_bps
            sizes = []
            while sum(sizes) < n_blk:
                nb = _XSIZES[len(sizes)] if len(sizes) < len(_XSIZES) else 4
                sizes.append(min(nb, n_blk - sum(sizes)))
            xdma = []
            b0 = 0
            for di, nb in enumerate(sizes):
                t = xpool.tile([128, 1024 * nb], f8, tag=f"xq_{di}",
                               name=f"xq{b0}", bufs=1)
                xdma.append((b0, nb, t))
                b0 += nb

            def xissue(di):
                bb, nb, t = xdma[di]
                nc.sync.dma_start(out=t,
                                  in_=xwin[:, bb * 1024:(bb + nb) * 1024])

            # first block split by partition halves: the first conv1 pair
            # only reads partitions 0-63, so it can start on the half-DMA
            # group 0 (tiles 0-2, bases 0/32/64, K2<=26) only reads
            # partitions 0-89: split the first DMA so it starts sooner
            t0_ = xdma[0][2]
            nc.sync.dma_start(out=t0_[0:96, :], in_=xwin[0:96, 0:1024])
            nc.sync.dma_start(out=t0_[96:128, :], in_=xwin[96:128, 0:1024])
            xissue(1)
            wcpack_sb = cpool.tile([128, n_wc], f8, tag="wcp",
                                   name="wcpack_sb")
            nc.sync.dma_start(out=wcpack_sb[:, 0:n_wc // 2],
                              in_=wcpack_d[:, 0:n_wc // 2])
            nc.sync.dma_start(out=wcpack_sb[:, n_wc // 2:n_wc],
                              in_=wcpack_d[:, n_wc // 2:n_wc])
            fw2t_sb = cpool.tile_from(fw2t_d[:], name="fw2t_sb")
            for di in range(2, len(sizes)):
                xissue(di)

            def quad_ap(sl, bi):
                blk = sl * n_bps + bi
                for (bb, nb, t) in xdma:
                    if bb <= blk < bb + nb:
                        return t[:, (blk - bb) * 1024:(blk - bb) * 1024 + 1024]
                raise AssertionError(blk)

            # z psum tiles (also the dummy-matmul target for the
            # single-sync-wait preamble)
            zs = [zps.tile([128, _SLICE], f32, tag="z", name=f"z{sl}")
                  for sl in range(_NSL)]

            # single-sync-wait rule: pre-observe PE-read const queues with
            # dummy 1-col matmuls; DVE/ACT-read consts with dummy touches.
            nc.tensor.matmul(zs[0][0:128, 0:1], w1stat_sb[0:26, 0:128],
                             w1stat_sb[0:26, 0:1], start=True, stop=True)
            dvescr = wpool.tile([128, 44], f32, tag="dvescr", name="dvescr",
                                bufs=1)
            nc.vector.tensor_copy(out=dvescr[:, 0:1], in_=biasp_sb[:])
            nc.vector.tensor_copy(out=dvescr[:, 4:44], in_=fb2r_sb[:])
            actscr = wpool.tile([128, 1], f32, tag="actscr", name="actscr",
                                bufs=1)
            nc.scalar.copy(out=actscr[:], in_=bcombN_sb[:])

            recip_insts = []
            for sl in range(_NSL):
                stiles = {}      # group -> s tile
                zu_next = 0
                for gi in range(n_grp):
                    gtiles = list(range(gi * _GRP,
                                        min((gi + 1) * _GRP, n_tiles)))
                    ng_t = len(gtiles)
                    cp = cps.tile([128, _GRP * _SLICE], f32, tag="cp",
                                  name=f"cp{sl}_{gi}")
                    for j, ti in enumerate(gtiles):
                        t = tiles[ti]
                        K2 = t["K2"]
                        bi, g = divmod(ti, 4)
                        quad = quad_ap(sl, bi)
                        rhs = quad[32 * g:32 * g + K2, :].rearrange(
                            "p (two n) -> p two n", two=2)
                        lhsT = w1stat_sb[
                            32 * g:32 * g + K2,
                            bi * 256:bi * 256 + 256].rearrange(
                            "p (two m) -> p two m", two=2)
                        nc.tensor.matmul(
                            cp[:, j * _SLICE:(j + 1) * _SLICE], lhsT,
                            rhs, start=True, stop=True, perf_mode=DR,
                            tile_position=(32 * g, 0))
                    q = qpool.tile([128, _GRP * _SLICE], f32, tag="q",
                                   name=f"q{sl}_{gi}")
                    s = spool.tile([128, _GRP * _SLICE], f8, tag="s",
                                   name=f"s{sl}_{gi}")
                    stiles[gi] = s
                    nc.vector._custom_dve(
                        den_op, out=q[:, 0:ng_t * _SLICE],
                        in0=cp[:, 0:ng_t * _SLICE],
                        s0=biasp_sb[0:128, 0:1], s1=1.0 / 16.0, imm2=2.0)
                    if sl == 0 and gi == 0:
                        # deferred const-queue dummies (single-sync-wait):
                        # emitted after group 0's conv1 so they park in the
                        # PE wait queue while their (later) DMAs land.
                        nc.tensor.matmul(zs[0][0:128, 0:1],
                                         wcpack_sb[0:128, 0:128],
                                         wcpack_sb[0:128, 0:1],
                                         start=True, stop=True)
                        nc.tensor.matmul(zs[0][0:10, 0:1],
                                         fw2t_sb[0:128, 0:10],
                                         fw2t_sb[0:128, 0:1],
                                         start=True, stop=True)
                    if sl == _NSL - 1 and gi == n_grp - 1:
                        # final group's reciprocal on the (otherwise idle)
                        # DVE so the ACT queue can start the table switch
                        # and tail while this group finishes
                        nc.vector._custom_dve(
                            RECIPROCAL_APPROX_FAST,
                            out=s[:, 0:ng_t * _SLICE],
                            in0=q[:, 0:ng_t * _SLICE],
                            s0=RC["s0"], s1=RC["s1"], imm2=RC["imm2"])
                    else:
                        ri = _act_raw(nc, s[:, 0:ng_t * _SLICE],
                                      q[:, 0:ng_t * _SLICE], AF.Reciprocal)
                        recip_insts.append(ri)
                    # z units whose tiles are all covered by groups <= gi
                    while zu_next < len(zunits):
                        kind, t0, col0 = zunits[zu_next]
                        t_last = t0 + (1 if kind == "dr" else 0)
                        if t_last > gtiles[-1]:
                            break
                        sg = stiles[t0 // _GRP]
                        o0 = (t0 % _GRP) * _SLICE
                        if kind == "dr":
                            lhsT = wcpack_sb[
                                :, col0:col0 + 256].rearrange(
                                "p (two m) -> p two m", two=2)
                            rhs = sg[:, o0:o0 + 1024].rearrange(
                                "p (two n) -> p two n", two=2)
                            nc.tensor.matmul(zs[sl], lhsT, rhs,
                                             start=(t0 == 0), stop=False,
                                             perf_mode=DR)
                        else:
                            lhsT = wcpack_sb[:, col0:col0 + 128]
                            rhs = sg[:, o0:o0 + _SLICE]
                            nc.tensor.matmul(
                                zs[sl], lhsT, rhs, start=(t0 == 0),
                                stop=(t_last == n_tiles - 1))
                        zu_next += 1

            # ---- tail: sigmoid via Exp + fast-reciprocal, fc2, log_softmax.
            # (no max-sub: |logits| < 12, exp cannot overflow fp32.)
            # NOTE: must stay after ALL recips - interleaving tail ACT ops
            # with recips thrashes the ACT table sets (measured 8 loads).
            last_recip = recip_insts[-1]
            for sl in range(_NSL):
                e = wpool.tile([128, _SLICE], f32, tag="e", name=f"e{sl}")
                ei = nc.scalar.activation(e, zs[sl], AF.Exp, bias=bcombN_sb[:],
                                          scale=-1.0 / 64.0)
                add_dep_helper(ei.ins, last_recip.ins, sync=False,
                               reason="keep tail ACT after recips (table sets)")
                t1 = wpool.tile([128, _SLICE], f32, tag="t1", name=f"t1{sl}")
                nc.vector.tensor_scalar_add(t1, e, 1.0)
                h = wpool.tile([128, _SLICE], f16, tag="h", name=f"h{sl}")
                nc.vector._custom_dve(RECIPROCAL_APPROX_FAST, out=h, in0=t1,
                                      s0=RC["s0"], s1=RC["s1"], imm2=RC["imm2"])
                ng = _SLICE // 128
                # fc2 psum borrows the cps pool (all 8 banks are spoken for;
                # the cp rotation is idle by the time the tail runs)
                fp = cps.tile([128, _GRP * _SLICE], f32, tag="cp",
                              name=f"fp{sl}")
                for g in range(ng):
                    nc.tensor.matmul(fp[:, g * 10:(g + 1) * 10],
                                     h[:, g * 128:(g + 1) * 128], fw2t_sb[:],
                                     start=True, stop=True)
                lg = wpool.tile([128, 10 * ng], f32, tag="lg", name=f"lg{sl}")
                nc.vector.tensor_tensor(out=lg, in0=fp[:, 0:10 * ng],
                                        in1=fb2r_sb[:, 0:10 * ng],
                                        op=AluOpType.add)
                e2 = wpool.tile([128, 10 * ng], f32, tag="e2", name=f"e2{sl}")
                e2i = nc.scalar.activation(e2, lg, AF.Exp)
                add_dep_helper(e2i.ins, last_recip.ins, sync=False,
                               reason="keep tail ACT after recips (table sets)")
                ssum = wpool.tile([128, ng], f32, tag="ss", name=f"ss{sl}")
                nc.vector.tensor_reduce(
                    ssum, e2.rearrange("p (g k) -> p g k", k=10),
                    axis=mybir.AxisListType.X, op=AluOpType.add)
                lns = wpool.tile([128, ng], f32, tag="ls", name=f"ls{sl}")
                li = nc.scalar.activation(lns, ssum, AF.Ln)
                add_dep_helper(li.ins, last_recip.ins, sync=False,
                               reason="keep tail ACT after recips (table sets)")
                ot = wpool.tile([128, 10 * ng], f32, tag="ot", name=f"ot{sl}")
                nc.vector.tensor_tensor(
                    out=ot.rearrange("p (g k) -> p g k", k=10),
                    in0=lg.rearrange("p (g k) -> p g k", k=10),
                    in1=lns.rearrange("p (g o) -> p g o", o=1).to_broadcast(
                        [128, ng, 10]),
                    op=AluOpType.subtract)
                orow = sl * _SLICE
                oeng = nc.sync if sl == 0 else nc.scalar
                oeng.dma_start(
                    out=out_d[orow:orow + _SLICE, :].rearrange(
                        "(g p) k -> p g k", p=128),
                    in_=ot.rearrange("p (g k) -> p g k", k=10))
    nc.compile()
    return nc


_PROGRAM_CACHE = {}


def kernel(x, w1, b1, w2, b2, fw1, fb1, fw2, fb2):
    global LAST_RESULTS
    tile_wins, consts, tiles = _host_prep(x, w1, b1, w2, b2, fw1, fb1, fw2, fb2)

    if "nc" not in _PROGRAM_CACHE:
        _PROGRAM_CACHE["nc"] = _build_program(tiles)
    nc = _PROGRAM_CACHE["nc"]

    f8 = ml_dtypes.float8_e4m3fn
    blocks = _blocks(tiles)
    n_bps = len(blocks)
    shared = {k: consts[k] for k in ("w1stat", "wcpack", "cb32", "fw2t")}
    in_maps = []
    for c in range(_NCORES):
        m = dict(shared)
        blob = np.zeros((128, _NSL * n_bps * 1024), f8)
        for sl in range(_NSL):
            for bi, blk in enumerate(blocks):
                col0 = (sl * n_bps + bi) * 1024
                for g, t_i in enumerate(blk):
                    t = tiles[t_i]
                    K2 = t["K2"]
                    w = tile_wins[t_i]            # [2, K2, B]
                    c0 = c * _PC + sl * _SLICE
                    for j in range(2):
                        blob[32 * g:32 * g + K2,
                             col0 + j * _SLICE:col0 + (j + 1) * _SLICE] = \
                            w[j, :, c0:c0 + _SLICE]
        m["xwin"] = blob
        in_maps.append(m)

    from concourse.bass_utils import run_bass_kernel_spmd
    trace = bool(int(os.environ.get("BASS_KERNEL_TRACE", "0")))
    res = run_bass_kernel_spmd(nc, in_maps, core_ids=list(range(_NCORES)),
                               trace=trace)
    LAST_RESULTS = res
    return np.concatenate([r["out"] for r in res.results], axis=0)
